# Initial kernel scaffold
#
"""Transformer block (pre-LN causal MHA + GELU MLP) on 8 trn2 NeuronCores.

Sharding: core r handles batch b=r//4, group position p=r%4, owning token
chunks {p, 7-p} of eight 256-token chunks (causally balanced zigzag).
Everything is sequence-parallel (zero duplicated flops) except attention:
K^T and V for the full batch are exchanged via AllGathers inside each
4-core batch group, split into two key-halves so attention on early keys
overlaps the second gather.

Attention computes transposed scores S^T[k, q] = K.Q^T so the softmax
row-sum falls out of a ones-augmented V matmul; no running max is needed
(|scores| <~ 7 for LN'd activations, exp is safe in fp32). Causal masks are
multiplicative 0/1 indicators built in-kernel from a tiny per-core qbase
input, so ONE SPMD program serves all 8 cores; head pairs are packed onto
the 128-partition axis (row-tiled K=64 matmuls) and the two phase-A query
chunks share 512-wide score/exp tiles.

Precision: LN + QKV run float32r (FP22, full PE rate); attention operands
(K/V/Q/exp/W_o) and the MLP (W_fc/h/W_fc2) are bf16 with fp32 PSUM
accumulation; LN gamma/beta are folded into the following weight matrix on
the host. Measured end-to-end relative error ~2.4e-3.

Self-contained: hardcodes B=2, T=2048, C=1024, H=16, D=64, hidden=4096.
"""
import sys

if "/opt/trn_rl_repo" not in sys.path:
    sys.path.insert(0, "/opt/trn_rl_repo")

import numpy as np
import ml_dtypes

B, T, C, H = 2, 2048, 1024, 16
D = C // H            # 64
MH = 4 * C            # 4096 mlp hidden
EPS = 1e-5
P = 128
TOK = 512             # tokens per core
NCH = 256             # tokens per chunk
N_CORES = 8
NEG = -1.0e9
SCALE = 1.0 / np.sqrt(D)

_CACHE: dict = {}


def _build(mock_cc=False):
    import concourse.tile as tile
    from concourse import bacc, mybir
    from concourse.masks import make_identity
    from contextlib import ExitStack

    F32 = mybir.dt.float32
    F32R = mybir.dt.float32r
    BF16 = mybir.dt.bfloat16
    I32 = mybir.dt.int32
    AF = mybir.ActivationFunctionType
    ALU = mybir.AluOpType

    nc = bacc.Bacc()

    # ---------------- I/O ----------------
    x_in = nc.declare_dram_parameter("x", [TOK, C], F32, isOutput=False)
    qbase_in = nc.declare_dram_parameter("qbase", [1, 2], F32, isOutput=False)
    w_attn = nc.declare_dram_parameter("w_attn", [C, 3 * C], F32, isOutput=False)
    b_attn = nc.declare_dram_parameter("b_attn", [3 * C], F32, isOutput=False)
    w_o = nc.declare_dram_parameter("w_o", [C, C], BF16, isOutput=False)
    b_o = nc.declare_dram_parameter("b_o", [C], F32, isOutput=False)
    w_fc = nc.declare_dram_parameter("w_fc", [C, MH], BF16, isOutput=False)
    b_fc = nc.declare_dram_parameter("b_fc", [MH], F32, isOutput=False)
    w_fc2 = nc.declare_dram_parameter("w_fc2", [MH, C], BF16, isOutput=False)
    b_fc2 = nc.declare_dram_parameter("b_fc2", [C], F32, isOutput=False)
    out_ext = nc.declare_dram_parameter("out", [TOK, C], F32, isOutput=True)

    # internal DRAM for the collectives (A = keys 0:1024, B = keys 1024:2048)
    kt_in = [nc.dram_tensor(f"kt_in_{s}", [C, NCH], BF16) for s in range(2)]
    v_in = [nc.dram_tensor(f"v_in_{s}", [NCH, C], BF16) for s in range(2)]
    kt_all = [nc.dram_tensor(f"kt_all_{s}", [4 * C, NCH], BF16) for s in range(2)]
    v_all = [nc.dram_tensor(f"v_all_{s}", [4 * NCH, C], BF16) for s in range(2)]
    stash_d = nc.dram_tensor("stash_d", [16, D + 1, NCH], F32)
    RG = [[0, 1, 2, 3], [4, 5, 6, 7]]

    def r32(ap):
        return ap.bitcast(F32R)

    with tile.TileContext(nc) as tc, ExitStack() as ctx:
        # ---------- pools: outer (whole kernel) ----------
        const = ctx.enter_context(tc.tile_pool(name="const", bufs=1))
        outer = ctx.enter_context(tc.tile_pool(name="outer", bufs=1))
        sm = ctx.enter_context(tc.tile_pool(name="sm", bufs=2))

        # ---------- constants ----------
        ident = const.tile([P, P], F32)
        make_identity(nc, ident)
        eps_t = const.tile([P, 1], F32)
        nc.vector.memset(eps_t, EPS)
        ones128 = const.tile([P, P], F32)
        nc.vector.memset(ones128, 1.0)
        ident_bf = const.tile([P, P], BF16)
        nc.vector.tensor_copy(out=ident_bf, in_=ident)

        # per-feature bias tiles [128, 1] views
        bq_sb = const.tile([P, 8], F32)     # b_attn[0:1024] -> [128, 8]
        nc.sync.dma_start(out=bq_sb, in_=b_attn[0:C].rearrange("(f p) -> p f", p=P))
        bk_sb = const.tile([P, 8], F32)
        nc.sync.dma_start(out=bk_sb, in_=b_attn[C:2 * C].rearrange("(f p) -> p f", p=P))
        bfc_sb = const.tile([P, 32], F32)
        nc.sync.dma_start(out=bfc_sb, in_=b_fc[:].rearrange("(f p) -> p f", p=P))
        # broadcast bias tiles [128, C]
        bv_bc = const.tile([P, C], F32)
        nc.sync.dma_start(out=bv_bc, in_=b_attn[2 * C:3 * C].rearrange("(a c) -> a c", a=1).to_broadcast((P, C)))
        bo_bc = const.tile([P, C], F32)
        nc.sync.dma_start(out=bo_bc, in_=b_o[:].rearrange("(a c) -> a c", a=1).to_broadcast((P, C)))
        b2_bc = const.tile([P, C], F32)
        nc.sync.dma_start(out=b2_bc, in_=b_fc2[:].rearrange("(a c) -> a c", a=1).to_broadcast((P, C)))

        # qbase + iotas for mask building
        qbase_sb = const.tile([1, 2], F32)
        nc.sync.dma_start(out=qbase_sb, in_=qbase_in[:, :])
        kidx_i = const.tile([P, 1], I32)
        nc.gpsimd.iota(kidx_i, pattern=[[0, 1]], base=0, channel_multiplier=1)
        kidx_f = const.tile([P, 1], F32)
        nc.vector.tensor_copy(out=kidx_f, in_=kidx_i)
        qio_i = const.tile([1, NCH], I32)
        nc.gpsimd.iota(qio_i, pattern=[[1, NCH]], base=0, channel_multiplier=0)
        qio_f = const.tile([1, NCH], F32)
        nc.vector.tensor_copy(out=qio_f, in_=qio_i)
        # qk[qc][k, q] = qglobal(qc, q) - k   (before subtracting 128*ktg)
        qk = []
        for qc in range(2):
            qg = const.tile([1, NCH], F32, name=f"qg{qc}")
            nc.vector.tensor_scalar_add(out=qg, in0=qio_f, scalar1=qbase_sb[0:1, qc:qc + 1])
            qgb = const.tile([P, NCH], F32, name=f"qgb{qc}")
            nc.gpsimd.partition_broadcast(qgb, qg)
            qkt = const.tile([P, NCH], F32, name=f"qk{qc}")
            nc.vector.tensor_scalar_sub(out=qkt, in0=qgb, scalar1=kidx_f)
            qk.append(qkt)

        # ---------- helpers ----------
        def layer_norm(src, dst_pool, tag, dt=F32):
            ln = dst_pool.tile([P, 4, C], dt, name=tag, tag=tag)
            for t in range(4):
                stats = sm.tile([P, 2, 6], F32, name="lnstats", tag="lnstats")
                nc.vector.bn_stats(out=stats[:, 0, :], in_=src[:, t, 0:512])
                nc.vector.bn_stats(out=stats[:, 1, :], in_=src[:, t, 512:1024])
                mv = sm.tile([P, 2], F32, name="lnmv", tag="lnmv")
                nc.vector.bn_aggr(out=mv, in_=stats)
                rstd = sm.tile([P, 1], F32, name="lnrstd", tag="lnrstd")
                nc.scalar.activation(out=rstd, in_=mv[:, 1:2], func=AF.Sqrt, bias=eps_t, scale=1.0)
                nc.vector.reciprocal(out=rstd, in_=rstd)
                nc.vector.tensor_scalar(out=ln[:, t, :], in0=src[:, t, :],
                                        scalar1=mv[:, 0:1], scalar2=rstd,
                                        op0=ALU.subtract, op1=ALU.mult)
            return ln

        def transpose_to(lnt, dst_pool, dst_tag, dt=F32R, idn=None):
            xt = dst_pool.tile([P, 8, TOK], dt, name=dst_tag, tag=dst_tag)
            with tc.tile_pool(name="tp_ps", bufs=2, space="PSUM") as tp_ps:
                for t in range(4):
                    for f in range(8):
                        pt = tp_ps.tile([P, P], lnt.dtype, name="tpt", tag="tpt",
                                        padded_shape=[P, 2 * P])
                        nc.tensor.transpose(pt[:, :], lnt[:, t, P * f:P * (f + 1)],
                                            idn if idn is not None else ident)
                        nc.vector.tensor_copy(out=xt[:, f, P * t:P * (t + 1)], in_=pt[:, :])
            return xt

        x2 = outer.tile([P, 4, C], F32)

        with tc.tile_pool(name="mid", bufs=1) as mid:
            x_sb = mid.tile([P, 4, C], F32)
            for t in range(4):
                nc.sync.dma_start(out=x_sb[:, t, :], in_=x_in[P * t:P * (t + 1), :])
            qT = mid.tile([P, 8, TOK], BF16)
            yT = mid.tile([P, 8, TOK], BF16)

            # ================= qkv =================
            with tc.tile_pool(name="qkvp", bufs=1) as qp, \
                 tc.tile_pool(name="wqkv", bufs=2) as wp, \
                 tc.tile_pool(name="qkv_ps", bufs=3, space="PSUM") as qkv_ps:
                ln1 = layer_norm(x_sb, qp, "ln")
                xlnT = transpose_to(ln1, qp, "xlnT")

                # K^T feature tiles -> kt_in halves
                for f in range(8):
                    if f % 4 == 0:
                        wk = wp.tile([P, 8, 512], F32R, name="wk", tag="wk")
                        nc.scalar.dma_start(out=wk, in_=r32(w_attn[:, C + 512 * (f // 4): C + 512 * (f // 4 + 1)]
                                                           .rearrange("(kc kp) n -> kp kc n", kp=P)))
                    fo = P * (f % 4)
                    ps = qkv_ps.tile([P, TOK], F32, name="kps", tag="qkvps")
                    for k in range(8):
                        nc.tensor.matmul(ps[:, :], wk[:, k, fo:fo + P], xlnT[:, k, :],
                                         start=(k == 0), stop=(k == 7))
                    kt_sb = sm.tile([P, TOK], BF16, name="kt_sb", tag="kt_sb", bufs=2)
                    nc.vector.tensor_scalar_add(out=kt_sb, in0=ps[:, :], scalar1=bk_sb[:, f:f + 1])
                    for s in range(2):
                        nc.sync.dma_start(out=kt_in[s][P * f:P * (f + 1), :],
                                          in_=kt_sb[:, NCH * s:NCH * (s + 1)])
                # V token tiles -> v_in halves (t-outer so the phase-A half
                # finishes after t=1 and the first AllGather can fire early)
                wvs = []
                for n in range(2):
                    wv = qp.tile([P, 8, 512], F32R, name=f"wv{n}", tag=f"wv{n}")
                    (nc.scalar if n == 0 else nc.sync).dma_start(out=wv, in_=r32(w_attn[:, 2 * C + 512 * n:2 * C + 512 * (n + 1)]
                                                       .rearrange("(kc kp) n -> kp kc n", kp=P)))
                    wvs.append(wv)
                for t in range(4):
                    for n in range(2):
                        ps = qkv_ps.tile([P, 512], F32, name="vps", tag="qkvps")
                        for k in range(8):
                            nc.tensor.matmul(ps[:, :], xlnT[:, k, P * t:P * (t + 1)],
                                             wvs[n][:, k, :], start=(k == 0), stop=(k == 7))
                        v_sb = sm.tile([P, 512], BF16, name="v_sb", tag="v_sb")
                        nc.vector.tensor_tensor(out=v_sb, in0=ps[:, :],
                                                in1=bv_bc[:, 512 * n:512 * (n + 1)], op=ALU.add)
                        sh, row = divmod(t, 2)
                        nc.sync.dma_start(out=v_in[sh][P * row:P * (row + 1), 512 * n:512 * (n + 1)],
                                          in_=v_sb)
                # collectives (gpsimd-triggered; overlap with Q^T compute below)
                for s in range(2):
                    if mock_cc:
                        nc.gpsimd.dma_start(out=kt_all[s][0:C, :], in_=kt_in[s][:, :])
                        nc.gpsimd.dma_start(out=v_all[s][0:NCH, :], in_=v_in[s][:, :])
                    else:
                        nc.gpsimd.collective_compute("AllGather", ALU.bypass,
                                                     ins=[kt_in[s][:, :]], outs=[kt_all[s][:, :]],
                                                     replica_groups=RG)
                        nc.gpsimd.collective_compute("AllGather", ALU.bypass,
                                                     ins=[v_in[s][:, :]], outs=[v_all[s][:, :]],
                                                     replica_groups=RG)

                # Q^T feature tiles (stay local); fold in 1/sqrt(d)
                for f in range(8):
                    if f % 4 == 0:
                        wq = wp.tile([P, 8, 512], F32R, name="wq", tag="wk")
                        nc.sync.dma_start(out=wq, in_=r32(w_attn[:, 512 * (f // 4): 512 * (f // 4 + 1)]
                                                           .rearrange("(kc kp) n -> kp kc n", kp=P)))
                    fo = P * (f % 4)
                    ps = qkv_ps.tile([P, TOK], F32, name="qps", tag="qkvps")
                    for k in range(8):
                        nc.tensor.matmul(ps[:, :], wq[:, k, fo:fo + P], xlnT[:, k, :],
                                         start=(k == 0), stop=(k == 7))
                    nc.vector.tensor_scalar(out=qT[:, f, :], in0=ps[:, :], scalar1=bq_sb[:, f:f + 1],
                                            scalar2=SCALE, op0=ALU.add, op1=ALU.mult)

            # ============ attention (+ proj overlapped into phase B) ============
            with tc.tile_pool(name="attp", bufs=1) as ap, \
                 tc.tile_pool(name="projp", bufs=1) as pp, \
                 tc.tile_pool(name="pr_ps", bufs=2, space="PSUM") as pr_ps:
                wo_sb = pp.tile([P, 8, C], BF16)
                nc.sync.dma_start(out=wo_sb, in_=w_o[:, :].rearrange("(kc kp) n -> kp kc n", kp=P))
                for t in range(4):
                    nc.vector.tensor_tensor(out=x_sb[:, t, :], in0=x_sb[:, t, :], in1=bo_bc, op=ALU.add)

                def load_kv(s):
                    ktb = ap.tile([P, 8, 4, NCH], BF16, name="ktb", tag="ktb", bufs=2)
                    vb = ap.tile([P, 8, 16, D + 1], BF16, name="vb", tag="vb", bufs=2)
                    for r in range(4):
                        blk = r if s == 0 else 3 - r     # rank block -> key slot
                        nc.sync.dma_start(
                            out=ktb[:, :, blk, :],
                            in_=kt_all[s][C * r:C * (r + 1), :].rearrange("(j p) c -> p j c", p=P))
                        for sub in range(2):
                            nc.sync.dma_start(
                                out=vb[:, 2 * blk + sub, :, 0:D],
                                in_=v_all[s][NCH * r + P * sub:NCH * r + P * (sub + 1), :]
                                        .rearrange("p (h d) -> p h d", h=H))
                    nc.vector.tensor_copy(out=vb[:, :, :, D:D + 1],
                                          in_=ones128.rearrange("p (a b) -> p a b", a=8)[:, :, 0:16])
                    return ktb, vb

                def build_ind(s, qc):
                    ind = sm.tile([P, 8, 2, NCH], BF16, name="ind", tag="ind", bufs=1)
                    for kt in range(8):
                        ktg = 8 * s + kt
                        for i in range(2):
                            nc.vector.tensor_scalar(out=ind[:, kt, i, :], in0=qk[qc],
                                                    scalar1=float(P * ktg), scalar2=None,
                                                    op0=ALU.is_ge)
                    return ind

                def div_write(ya_h, h, j, qc, ysrc, rsrc):
                    recip = sm.tile([1, NCH], F32, name=f"rc{h}", tag=f"rc{h}")
                    nc.vector.reciprocal(out=recip, in_=rsrc)
                    rb = sm.tile([D, NCH], F32, name=f"rb{h}", tag=f"rb{h}")
                    nc.gpsimd.partition_broadcast(rb, recip)
                    nc.vector.tensor_tensor(out=yT[64 * h:64 * (h + 1), j, NCH * qc:NCH * (qc + 1)],
                                            in0=ysrc, in1=rb, op=ALU.mult)

                def proj(trange):
                    for t in trange:
                        for n in range(2):
                            ps = pr_ps.tile([P, 512], F32, name="prps", tag="prps")
                            for k in range(8):
                                nc.tensor.matmul(ps[:, :], yT[:, k, P * t:P * (t + 1)],
                                                 wo_sb[:, k, 512 * n:512 * (n + 1)],
                                                 start=(k == 0), stop=(k == 7))
                            nc.vector.tensor_tensor(out=x2[:, t, 512 * n:512 * (n + 1)], in0=ps[:, :],
                                                    in1=x_sb[:, t, 512 * n:512 * (n + 1)], op=ALU.add)

                # ---- load/build both phases up front: phase-B tiles land while
                # phase-A computes (hides the second AllGather + load bubble) ----
                ktb, vb = load_kv(0)
                ind = build_ind(0, 0)     # only chunk 0 can be non-causal here
                ktbB, vbB = load_kv(1)
                with tc.tile_pool(name="at_ps0", bufs=1, space="PSUM") as at_ps:
                    for j in range(8):
                        ya = [at_ps.tile([D + 1, TOK], F32, name=f"ya{h}", tag=f"ya{h}", bufs=1)
                              for h in range(2)]
                        for kt in range(8):
                            st = at_ps.tile([P, 2, TOK], F32, name="st", tag="st", bufs=2)
                            for h in range(2):
                                nc.tensor.matmul(
                                    st[:, h, :],
                                    ktb[64 * h:64 * (h + 1), j, kt // 2, (kt % 2) * P:(kt % 2) * P + P],
                                    qT[64 * h:64 * (h + 1), j, :],
                                    start=True, stop=True, tile_position=(64 * h, 0))
                            et = sm.tile([P, 2, TOK], BF16, name="et", tag="et", bufs=3)
                            nc.scalar.activation(out=et, in_=st[:, :, :], func=AF.Exp, scale=1.0)
                            nc.vector.tensor_tensor(out=et[:, :, 0:NCH], in0=et[:, :, 0:NCH],
                                                    in1=ind[:, kt, :, :], op=ALU.mult)
                            for h in range(2):
                                nc.tensor.matmul(ya[h][:, :], vb[:, kt, 2 * j + h, :], et[:, h, :],
                                                 start=(kt == 0), stop=(kt == 7))
                        for h in range(2):
                            hh = 2 * j + h
                            stc = sm.tile([D + 1, NCH], F32, name=f"stc{h}", tag=f"stc{h}")
                            nc.vector.tensor_copy(out=stc, in_=ya[h][:, NCH:TOK])
                            nc.sync.dma_start(out=stash_d[hh, :, :], in_=stc)
                            div_write(ya, h, j, 0, ya[h][0:D, 0:NCH], ya[h][D:D + 1, 0:NCH])

                proj([0, 1])

                # ---- phase B: keys 1024:2047, chunk 1 only ----
                ktb, vb = ktbB, vbB
                ind = build_ind(1, 1)
                with tc.tile_pool(name="at_ps1", bufs=1, space="PSUM") as at_ps:
                    for j in range(8):
                        ya = [at_ps.tile([D + 1, NCH], F32, name=f"ya{h}", tag=f"ya{h}", bufs=1)
                              for h in range(2)]
                        for kt in range(8):
                            st = at_ps.tile([P, 2, TOK], F32, name="st", tag="st", bufs=2)
                            for h in range(2):
                                nc.tensor.matmul(
                                    st[:, h, 0:NCH],
                                    ktb[64 * h:64 * (h + 1), j, kt // 2, (kt % 2) * P:(kt % 2) * P + P],
                                    qT[64 * h:64 * (h + 1), j, NCH:TOK],
                                    start=True, stop=True, tile_position=(64 * h, 0))
                            et = sm.tile([P, 2, NCH], BF16, name="etb", tag="etb", bufs=3)
                            nc.scalar.activation(out=et, in_=st[:, :, 0:NCH], func=AF.Exp, scale=1.0)
                            nc.vector.tensor_tensor(out=et[:, :, :], in0=et[:, :, :],
                                                    in1=ind[:, kt, :, :], op=ALU.mult)
                            for h in range(2):
                                nc.tensor.matmul(ya[h][:, :], vb[:, kt, 2 * j + h, :], et[:, h, :],
                                                 start=(kt == 0), stop=(kt == 7))
                        for h in range(2):
                            hh = 2 * j + h
                            stl = sm.tile([D + 1, NCH], F32, name=f"stl{h}", tag=f"stl{h}", bufs=1)
                            nc.sync.dma_start(out=stl, in_=stash_d[hh, :, :])
                            ysum = sm.tile([D + 1, NCH], F32, name=f"ys{h}", tag=f"ys{h}")
                            nc.vector.tensor_tensor(out=ysum, in0=ya[h][:, :], in1=stl, op=ALU.add)
                            div_write(ya, h, j, 1, ysum[0:D, :], ysum[D:D + 1, :])

                proj([2, 3])

        # ================= LN2 + MLP =================
        with tc.tile_pool(name="mlpp", bufs=1) as mp, \
             tc.tile_pool(name="wmlp", bufs=3) as wmp:
            ln2 = layer_norm(x2, mp, "ln2", dt=BF16)
            xln2T = transpose_to(ln2, mp, "xln2T", dt=BF16, idn=ident_bf)
            for t in range(4):
                nc.vector.tensor_tensor(out=x2[:, t, :], in0=x2[:, t, :], in1=b2_bc, op=ALU.add)

            h_sb = mp.tile([P, 32, 512], BF16)
            for half in range(2):
                with tc.tile_pool(name=f"mlp_ps{half}", bufs=1, space="PSUM") as mlp_ps:
                    ops = [mlp_ps.tile([P, 512], F32, name=f"ops{t}", tag=f"ops{t}", bufs=1)
                           for t in range(4)]
                    for m in range(32):
                        if half == 0:
                            if m % 4 == 0:
                                wfc = wmp.tile([P, 8, 512], BF16, name="wfc", tag="wfc")
                                nc.sync.dma_start(out=wfc,
                                                    in_=w_fc[:, 512 * (m // 4):512 * (m // 4 + 1)]
                                                    .rearrange("(kc kp) n -> kp kc n", kp=P))
                            mo = P * (m % 4)
                            fps = mlp_ps.tile([P, 512], F32, name="fps", tag="fps", bufs=4)
                            for k in range(8):
                                nc.tensor.matmul(fps[:, :], wfc[:, k, mo:mo + P], xln2T[:, k, :],
                                                 start=(k == 0), stop=(k == 7))
                            nc.scalar.activation(out=h_sb[:, m, :], in_=fps[:, :], func=AF.Gelu,
                                                 bias=bfc_sb[:, m:m + 1], scale=1.0)
                        if m % 4 == 0:
                            w2 = wmp.tile([P, 4, 512], BF16, name="w2", tag="w2", bufs=3)
                            nc.scalar.dma_start(out=w2, in_=w_fc2[P * m:P * (m + 4),
                                                               512 * half:512 * (half + 1)]
                                                .rearrange("(mc mp) n -> mp mc n", mp=P))
                        for t in range(4):
                            nc.tensor.matmul(ops[t][:, :], h_sb[:, m, P * t:P * (t + 1)],
                                             w2[:, m % 4, :], start=(m == 0), stop=(m == 31))
                    for t in range(4):
                        nc.vector.tensor_tensor(out=x2[:, t, 512 * half:512 * (half + 1)],
                                                in0=ops[t][:, :],
                                                in1=x2[:, t, 512 * half:512 * (half + 1)], op=ALU.add)
                        if half == 1:
                            nc.sync.dma_start(out=out_ext[P * t:P * (t + 1), :], in_=x2[:, t, :])

    nc.finalize()
    return nc


def _get_nc():
    if "nc" not in _CACHE:
        _CACHE["nc"] = _build()
    return _CACHE["nc"]


def _prep(**inputs):
    f = lambda a: np.asarray(a, dtype=np.float32)
    x = f(inputs["x"])
    ln1_g, ln1_b = f(inputs["ln1_g"]), f(inputs["ln1_b"])
    ln2_g, ln2_b = f(inputs["ln2_g"]), f(inputs["ln2_b"])
    W_attn, b_attn = f(inputs["W_attn"]), f(inputs["b_attn"])
    W_o, b_o = f(inputs["W_o"]), f(inputs["b_o"])
    W_fc, b_fc = f(inputs["W_fc"]), f(inputs["b_fc"])
    W_fc2, b_fc2 = f(inputs["W_fc2"]), f(inputs["b_fc2"])

    # fold LN affine params into the next matmul
    W_attn_e = ln1_g[:, None] * W_attn
    b_attn_e = b_attn + ln1_b @ W_attn
    W_fc_e = ln2_g[:, None] * W_fc
    b_fc_e = b_fc + ln2_b @ W_fc

    in_maps = []
    for r in range(N_CORES):
        b, p = divmod(r, 4)
        c0, c1 = p, 7 - p
        xs = np.concatenate([x[b, NCH * c0:NCH * (c0 + 1)],
                             x[b, NCH * c1:NCH * (c1 + 1)]], axis=0)
        in_maps.append({
            "x": np.ascontiguousarray(xs),
            "qbase": np.array([[NCH * c0, NCH * c1]], dtype=np.float32),
            "w_attn": W_attn_e, "b_attn": b_attn_e,
            "w_o": W_o.astype(ml_dtypes.bfloat16), "b_o": b_o,
            "w_fc": W_fc_e.astype(ml_dtypes.bfloat16), "b_fc": b_fc_e,
            "w_fc2": W_fc2.astype(ml_dtypes.bfloat16), "b_fc2": b_fc2,
        })

    def assemble(results):
        out = np.empty((B, T, C), dtype=np.float32)
        for r in range(N_CORES):
            b, p = divmod(r, 4)
            c0, c1 = p, 7 - p
            o = results[r]["out"]
            out[b, NCH * c0:NCH * (c0 + 1)] = o[0:NCH]
            out[b, NCH * c1:NCH * (c1 + 1)] = o[NCH:TOK]
        return out

    return in_maps, assemble


def kernel(**inputs):
    from concourse.bass_utils import run_bass_kernel_spmd

    in_maps, assemble = _prep(**inputs)
    res = run_bass_kernel_spmd(_get_nc(), in_maps, list(range(N_CORES)))
    return assemble(res.results)



# revision 11
# speedup vs baseline: 1.4592x; 1.4592x over previous
"""Transformer block (pre-LN causal MHA + GELU MLP) on 8 trn2 NeuronCores.

Sharding: core r handles batch b=r//4, group position p=r%4, owning token
chunks {p, 7-p} of eight 256-token chunks (causally balanced zigzag).
Sequence-parallel everywhere except attention: K^T and V for the full batch
are exchanged via fp8 AllGathers inside each 4-core batch group.

All heavy matmuls run in fp8e4 with DoubleRow perf mode (2 contraction
k-tiles per instruction at 0.5 cycles/row): QKV projections, attention
scores (K=64 with a zeroed second subtile on the Q side), attention*V
(key-tile pairs), output projection, and the MLP (precision tier
selectable per matmul via the FP8_* flags).

Masking is done on the PE + Act engines instead of element-wise DVE
multiplies: fully-masked (key-block, chunk) tiles get exp bias -30 from a
data-driven per-tile bias table (exp underflows to 0 in fp8), and the two
diagonal key-blocks per chunk get -256 added to the masked triangle via a
single extra matmul (lhsT=-256*I, rhs=triangle indicator built from qbase)
before the exp. Scores carry no 1/sqrt(d) or softmax-max handling: the
scale (0.125) and a -4*ln2 range shift are folded into the exp activation
(exp output ~ exp(s)/16 stays within fp8e4 range; the shift cancels in the
softmax division).

Bias handling: K bias is dropped (softmax is invariant to per-query score
shifts), V bias is folded into the residual bias on the host
(b_o + b_v @ W_o), Q bias is applied on the PSUM->SBUF copy, fc bias rides
the GELU activation, fc2 bias is pre-added to the residual.

LN rsqrt = exp(-0.5*ln(var+eps)) so LN1/attention/LN2 share one activation
table (natural_log_exp) and only the MLP's gelu forces a table switch.

Self-contained: hardcodes B=2, T=2048, C=1024, H=16, D=64, hidden=4096.
"""
import sys

if "/opt/trn_rl_repo" not in sys.path:
    sys.path.insert(0, "/opt/trn_rl_repo")

import numpy as np
import ml_dtypes

B, T, C, H = 2, 2048, 1024, 16
D = C // H            # 64
MH = 4 * C            # 4096 mlp hidden
EPS = 1e-5
P = 128
TOK = 512             # tokens per core
NCH = 256             # tokens per chunk
N_CORES = 8
EXPB = -2.7725887     # -4*ln2: exp emits exp(s)/16
SCALE = 0.125         # 1/sqrt(D)

# precision tiers (fp8 DoubleRow vs bf16) — tuned empirically
FP8_FC1 = True
FP8_FC2 = True

FP8 = ml_dtypes.float8_e4m3

_CACHE: dict = {}


def _build(mock_cc=False):
    import concourse.tile as tile
    from concourse import bacc, mybir
    from concourse.masks import make_identity
    from contextlib import ExitStack

    F32 = mybir.dt.float32
    BF16 = mybir.dt.bfloat16
    FP8D = mybir.dt.float8e4
    I32 = mybir.dt.int32
    AF = mybir.ActivationFunctionType
    ALU = mybir.AluOpType
    DR = mybir.MatmulPerfMode.DoubleRow

    FC1D = FP8D if FP8_FC1 else BF16
    FC2D = FP8D if FP8_FC2 else BF16

    nc = bacc.Bacc()

    # ---------------- I/O ----------------
    x_in = nc.declare_dram_parameter("x", [TOK, C], F32, isOutput=False)
    qbase_in = nc.declare_dram_parameter("qbase", [1, 2], F32, isOutput=False)
    wq_d = nc.declare_dram_parameter("wq", [C, C], FP8D, isOutput=False)
    wk_d = nc.declare_dram_parameter("wk", [C, C], FP8D, isOutput=False)
    wv_d = nc.declare_dram_parameter("wv", [C, C], FP8D, isOutput=False)
    bq_d = nc.declare_dram_parameter("bq", [C], F32, isOutput=False)
    wo_d = nc.declare_dram_parameter("wo", [C, C], FP8D, isOutput=False)
    rb_d = nc.declare_dram_parameter("rb", [C], F32, isOutput=False)
    wfc_d = nc.declare_dram_parameter("w_fc", [C, MH], FC1D, isOutput=False)
    bfc_d = nc.declare_dram_parameter("b_fc", [MH], F32, isOutput=False)
    wfc2_d = nc.declare_dram_parameter("w_fc2", [MH, C], FC2D, isOutput=False)
    bfc2_d = nc.declare_dram_parameter("b_fc2", [C], F32, isOutput=False)
    out_ext = nc.declare_dram_parameter("out", [TOK, C], F32, isOutput=True)

    # internal DRAM for the collectives (half s=0: keys 0:1024, s=1: 1024:2048)
    kt_in = [nc.dram_tensor(f"kt_in_{s}", [C, NCH], FP8D) for s in range(2)]
    v_in = [nc.dram_tensor(f"v_in_{s}", [NCH, C], FP8D) for s in range(2)]
    kt_all = [nc.dram_tensor(f"kt_all_{s}", [4 * C, NCH], FP8D) for s in range(2)]
    v_all = [nc.dram_tensor(f"v_all_{s}", [4 * NCH, C], FP8D) for s in range(2)]
    stash_d = nc.dram_tensor("stash_d", [16, D + 1, NCH], F32)
    RG = [[0, 1, 2, 3], [4, 5, 6, 7]]

    with tile.TileContext(nc) as tc, ExitStack() as ctx:
        # ---------- pools: outer (whole kernel) ----------
        const = ctx.enter_context(tc.tile_pool(name="const", bufs=1))
        outer = ctx.enter_context(tc.tile_pool(name="outer", bufs=1))
        sm = ctx.enter_context(tc.tile_pool(name="sm", bufs=2))

        # ---------- constants ----------
        ident = const.tile([P, P], F32)
        make_identity(nc, ident)
        ident_bf = const.tile([P, P], BF16)
        nc.vector.tensor_copy(out=ident_bf, in_=ident)
        eps_t = const.tile([P, 1], F32)
        nc.vector.memset(eps_t, EPS)
        ones128 = const.tile([P, P], F32)
        nc.vector.memset(ones128, 1.0)
        # -256 * I in fp8 (tri-mask stationary operand)
        negI = const.tile([P, P], FP8D)
        negI_f = const.tile([P, P], F32)
        nc.vector.tensor_scalar(out=negI_f, in0=ident, scalar1=-256.0, scalar2=None,
                                op0=ALU.mult)
        nc.vector.tensor_copy(out=negI, in_=negI_f)

        bq_sb = const.tile([P, 8], F32)     # q bias -> [128, 8]
        nc.sync.dma_start(out=bq_sb, in_=bq_d[0:C].rearrange("(f p) -> p f", p=P))
        bfc_sb = const.tile([P, 32], F32)
        nc.sync.dma_start(out=bfc_sb, in_=bfc_d[:].rearrange("(f p) -> p f", p=P))
        rb_bc = const.tile([P, C], F32)     # residual bias (b_o + b_v@W_o) bcast
        nc.sync.dma_start(out=rb_bc, in_=rb_d[:].rearrange("(a c) -> a c", a=1).to_broadcast((P, C)))
        b2_bc = const.tile([P, C], F32)
        nc.sync.dma_start(out=b2_bc, in_=bfc2_d[:].rearrange("(a c) -> a c", a=1).to_broadcast((P, C)))

        # qbase + iotas for mask tables
        qbase_sb = const.tile([1, 2], F32)
        nc.sync.dma_start(out=qbase_sb, in_=qbase_in[:, :])
        kidx_i = const.tile([P, 1], I32)
        nc.gpsimd.iota(kidx_i, pattern=[[0, 1]], base=0, channel_multiplier=1)
        kidx_f = const.tile([P, 1], F32)
        nc.vector.tensor_copy(out=kidx_f, in_=kidx_i)
        qio_i = const.tile([1, P], I32)
        nc.gpsimd.iota(qio_i, pattern=[[1, P]], base=0, channel_multiplier=0)
        qio_f = const.tile([1, P], F32)
        nc.vector.tensor_copy(out=qio_f, in_=qio_i)
        # TRI[k, q] = 1 if q < k else 0  (masked region of an aligned 128-diag)
        qio_bc = const.tile([P, P], F32)
        nc.gpsimd.partition_broadcast(qio_bc, qio_f)
        tri_f = const.tile([P, P], F32)
        nc.vector.tensor_scalar(out=tri_f, in0=qio_bc, scalar1=kidx_f, scalar2=None,
                                op0=ALU.is_lt)

        # ---- per-(phase, kt, chunk) exp bias table: alive -> EXPB, dead -> -30
        # slot order: (s, kt, c) -> 32 slots (s in 0..1, kt 0..7, c 0..1)
        kb_i = const.tile([1, 32], I32)
        nc.gpsimd.iota(kb_i, pattern=[[1024, 2], [128, 8], [0, 2]], base=0,
                       channel_multiplier=0)
        kb_f = const.tile([1, 32], F32)
        nc.vector.tensor_copy(out=kb_f, in_=kb_i)
        csel_i = const.tile([1, 32], I32)   # 0,1,0,1,... chunk selector
        nc.gpsimd.iota(csel_i, pattern=[[0, 2], [0, 8], [1, 2]], base=0,
                       channel_multiplier=0)
        csel_f = const.tile([1, 32], F32)
        nc.vector.tensor_copy(out=csel_f, in_=csel_i)
        # qb_slot = qbase[c0] + csel*(qbase[c1]-qbase[c0])
        qdiff = const.tile([1, 1], F32)
        nc.vector.tensor_scalar(out=qdiff, in0=qbase_sb[0:1, 1:2],
                                scalar1=qbase_sb[0:1, 0:1], scalar2=None,
                                op0=ALU.subtract)
        qb_slot = const.tile([1, 32], F32)
        nc.vector.tensor_scalar(out=qb_slot, in0=csel_f, scalar1=qdiff,
                                scalar2=qbase_sb[0:1, 0:1], op0=ALU.mult, op1=ALU.add)
        # alive = (qb_slot + 255 >= kb)  <=>  qb_slot - kb >= -255
        alive = const.tile([1, 32], F32)
        nc.vector.tensor_tensor(out=alive, in0=qb_slot, in1=kb_f, op=ALU.subtract)
        nc.vector.tensor_scalar(out=alive, in0=alive, scalar1=-255.0, scalar2=None,
                                op0=ALU.is_ge)
        be_row = const.tile([1, 32], F32)   # -30 + alive*(30+EXPB)
        nc.vector.tensor_scalar(out=be_row, in0=alive, scalar1=30.0 + EXPB,
                                scalar2=-30.0, op0=ALU.mult, op1=ALU.add)
        be = const.tile([P, 32], F32)
        nc.gpsimd.partition_broadcast(be, be_row)

        # ---- tri-mask rhs table: mrhs[:, slot, :] = TRI * diag(slot)
        # diag(slot) = 1 iff kb[slot] == qb_slot + 128*parity(kt)
        par_i = const.tile([1, 32], I32)
        nc.gpsimd.iota(par_i, pattern=[[0, 2], [0, 4], [128, 2], [0, 2]], base=0,
                       channel_multiplier=0)   # (s, ktpair, par, c) -> 128*(kt%2)
        par_f = const.tile([1, 32], F32)
        nc.vector.tensor_copy(out=par_f, in_=par_i)
        dfl = const.tile([1, 32], F32)
        nc.vector.tensor_tensor(out=dfl, in0=kb_f, in1=par_f, op=ALU.subtract)
        nc.vector.tensor_tensor(out=dfl, in0=dfl, in1=qb_slot, op=ALU.is_equal)
        dflb = const.tile([P, 32], F32)
        nc.gpsimd.partition_broadcast(dflb, dfl)
        mrhs = const.tile([P, 32, P], FP8D)
        for sl in range(32):
            nc.vector.tensor_scalar(out=mrhs[:, sl, :], in0=tri_f,
                                    scalar1=dflb[:, sl:sl + 1], scalar2=None,
                                    op0=ALU.mult)
        # full-kill pattern for the odd diagonal block's dead first q-half
        mkill = const.tile([P, 32, P], FP8D)
        for sl in range(32):
            if (sl // 2) % 2 == 1:   # odd kt slots only
                nc.vector.tensor_scalar(out=mkill[:, sl, :], in0=ones128,
                                        scalar1=dflb[:, sl:sl + 1], scalar2=None,
                                        op0=ALU.mult)

        def slot(s, kt, c):
            return s * 16 + kt * 2 + c

        # ---------- helpers ----------
        def layer_norm(src, dst_pool, tag, dt=BF16):
            ln = dst_pool.tile([P, 4, C], dt, name=tag, tag=tag)
            for t in range(4):
                stats = sm.tile([P, 2, 6], F32, name="lnstats", tag="lnstats")
                nc.vector.bn_stats(out=stats[:, 0, :], in_=src[:, t, 0:512])
                nc.vector.bn_stats(out=stats[:, 1, :], in_=src[:, t, 512:1024])
                mv = sm.tile([P, 2], F32, name="lnmv", tag="lnmv")
                nc.vector.bn_aggr(out=mv, in_=stats)
                lnv = sm.tile([P, 1], F32, name="lnv", tag="lnv")
                nc.scalar.activation(out=lnv, in_=mv[:, 1:2], func=AF.Ln,
                                     bias=eps_t, scale=1.0)
                rstd = sm.tile([P, 1], F32, name="lnrstd", tag="lnrstd")
                nc.scalar.activation(out=rstd, in_=lnv, func=AF.Exp, scale=-0.5)
                nc.vector.tensor_scalar(out=ln[:, t, :], in0=src[:, t, :],
                                        scalar1=mv[:, 0:1], scalar2=rstd,
                                        op0=ALU.subtract, op1=ALU.mult)
            return ln

        def transpose_to(lnt, dst_pool, dst_tag, dt, idn):
            xt = dst_pool.tile([P, 8, TOK], dt, name=dst_tag, tag=dst_tag)
            with tc.tile_pool(name="tp_ps", bufs=2, space="PSUM") as tp_ps:
                for t in range(4):
                    for f in range(8):
                        pt = tp_ps.tile([P, P], lnt.dtype, name="tpt", tag="tpt",
                                        padded_shape=[P, 2 * P])
                        nc.tensor.transpose(pt[:, :], lnt[:, t, P * f:P * (f + 1)], idn)
                        nc.vector.tensor_copy(out=xt[:, f, P * t:P * (t + 1)], in_=pt[:, :])
            return xt

        x2 = outer.tile([P, 4, C], F32)
        yT = outer.tile([P, 8, TOK], FP8D)

        with tc.tile_pool(name="mid", bufs=1) as mid:
            x_sb = mid.tile([P, 4, C], F32)
            for t in range(4):
                nc.sync.dma_start(out=x_sb[:, t, :], in_=x_in[P * t:P * (t + 1), :])
            # qz: [part, sub(2), j, tok]  sub1 = zeros (DoubleRow zero-subtile)
            qz = mid.tile([P, 2, 8, TOK], FP8D)
            nc.gpsimd.memset(qz[:, 1, :, :], 0.0)

            # ================= qkv =================
            with tc.tile_pool(name="qkvp", bufs=1) as qp, \
                 tc.tile_pool(name="wqkv", bufs=2) as wp:
                ln1 = layer_norm(x_sb, qp, "ln")
                xT8 = transpose_to(ln1, qp, "xT8", FP8D, ident_bf)
                qkv_ps_cm = tc.tile_pool(name="qkv_ps", bufs=3, space="PSUM")
                qkv_ps = qkv_ps_cm.__enter__()

                wk_sb = wp.tile([P, 8, C], FP8D, name="wk", tag="wk")
                nc.scalar.dma_start(out=wk_sb, in_=wk_d[:, :].rearrange("(kc kp) n -> kp kc n", kp=P))
                # K^T feature tiles -> kt_in halves (K bias dropped: softmax-invariant)
                for f in range(8):
                    fo = P * f
                    ps = qkv_ps.tile([P, TOK], F32, name="kps", tag="qkvps")
                    for k in range(4):
                        for hh in range(2):
                            nc.tensor.matmul(ps[:, TOK // 2 * hh:TOK // 2 * (hh + 1)],
                                             wk_sb[:, 2 * k:2 * k + 2, fo:fo + P],
                                             xT8[:, 2 * k:2 * k + 2, 256 * hh:256 * (hh + 1)],
                                             start=(k == 0), stop=(k == 3), perf_mode=DR)
                    kt_sb = sm.tile([P, TOK], FP8D, name="kt_sb", tag="kt_sb", bufs=2)
                    nc.vector.tensor_copy(out=kt_sb, in_=ps[:, :])
                    for s in range(2):
                        nc.sync.dma_start(out=kt_in[s][P * f:P * (f + 1), :],
                                          in_=kt_sb[:, NCH * s:NCH * (s + 1)])
                # V token tiles -> v_in halves (V bias folded into residual bias)
                wv_sb = wp.tile([P, 8, C], FP8D, name="wv", tag="wk")
                nc.sync.dma_start(out=wv_sb, in_=wv_d[:, :].rearrange("(kc kp) n -> kp kc n", kp=P))
                for t in range(4):
                    ps = qkv_ps.tile([P, C], F32, name="vps", tag="vps", bufs=2)
                    for k in range(4):
                        for n in range(4):
                            nc.tensor.matmul(ps[:, NCH * n:NCH * (n + 1)],
                                             xT8[:, 2 * k:2 * k + 2, P * t:P * (t + 1)],
                                             wv_sb[:, 2 * k:2 * k + 2, NCH * n:NCH * (n + 1)],
                                             start=(k == 0), stop=(k == 3), perf_mode=DR)
                    v_sb = sm.tile([P, C], FP8D, name="v_sb", tag="v_sb")
                    nc.vector.tensor_copy(out=v_sb, in_=ps[:, :])
                    sh, row = divmod(t, 2)
                    nc.sync.dma_start(out=v_in[sh][P * row:P * (row + 1), :], in_=v_sb)
                # collectives (overlap with Q^T compute below)
                for s in range(2):
                    if mock_cc:
                        nc.gpsimd.dma_start(out=kt_all[s][0:C, :], in_=kt_in[s][:, :])
                        nc.gpsimd.dma_start(out=v_all[s][0:NCH, :], in_=v_in[s][:, :])
                    else:
                        nc.gpsimd.collective_compute("AllGather", ALU.bypass,
                                                     ins=[kt_in[s][:, :]], outs=[kt_all[s][:, :]],
                                                     replica_groups=RG)
                        nc.gpsimd.collective_compute("AllGather", ALU.bypass,
                                                     ins=[v_in[s][:, :]], outs=[v_all[s][:, :]],
                                                     replica_groups=RG)

                # Q^T feature tiles (stay local); bias on copy, scale folded in exp
                wq_sb = wp.tile([P, 8, C], FP8D, name="wq", tag="wk")
                nc.sync.dma_start(out=wq_sb, in_=wq_d[:, :].rearrange("(kc kp) n -> kp kc n", kp=P))
                for f in range(8):
                    fo = P * f
                    ps = qkv_ps.tile([P, TOK], F32, name="qps", tag="qkvps")
                    for k in range(4):
                        for hh in range(2):
                            nc.tensor.matmul(ps[:, TOK // 2 * hh:TOK // 2 * (hh + 1)],
                                             wq_sb[:, 2 * k:2 * k + 2, fo:fo + P],
                                             xT8[:, 2 * k:2 * k + 2, 256 * hh:256 * (hh + 1)],
                                             start=(k == 0), stop=(k == 3), perf_mode=DR)
                    nc.vector.tensor_scalar(out=qz[:, 0, f, :], in0=ps[:, :],
                                            scalar1=bq_sb[:, f:f + 1], scalar2=None,
                                            op0=ALU.add)
                qkv_ps_cm.__exit__(None, None, None)

            # ============ attention (+ proj overlapped into phase B) ============
            with tc.tile_pool(name="attp", bufs=1) as ap, \
                 tc.tile_pool(name="projp", bufs=1) as pp, \
                 tc.tile_pool(name="pr_ps", bufs=2, space="PSUM") as pr_ps:
                wo_sb = pp.tile([P, 8, C], FP8D)
                nc.sync.dma_start(out=wo_sb, in_=wo_d[:, :].rearrange("(kc kp) n -> kp kc n", kp=P))
                for t in range(4):
                    nc.vector.tensor_tensor(out=x_sb[:, t, :], in0=x_sb[:, t, :], in1=rb_bc, op=ALU.add)

                def load_kv(s):
                    # ktb: [part(2h d), kt-slot(8+1 pad), j, keys]
                    ktb = ap.tile([P, 9, 8, P], FP8D, name="ktb", tag="ktb", bufs=2)
                    nc.gpsimd.memset(ktb[:, 8, :, :], 0.0)
                    # vb: [part(key), kt-slot, hh, D+1]
                    vb = ap.tile([P, 8, 16, D + 1], FP8D, name="vb", tag="vb", bufs=2)
                    nc.vector.tensor_copy(out=vb[:, :, :, D:D + 1],
                                          in_=ones128.rearrange("p (a b) -> p a b", a=8)[:, :, 0:16])
                    for r in range(4):
                        nc.sync.dma_start(
                            out=ktb[:, 2 * r:2 * r + 2, :, :],
                            in_=kt_all[s][C * r:C * (r + 1), :].rearrange(
                                "(j p) (kb kc) -> p kb j kc", p=P, kb=2))
                        for sub in range(2):
                            nc.sync.dma_start(
                                out=vb[:, 2 * r + sub, :, 0:D],
                                in_=v_all[s][NCH * r + P * sub:NCH * r + P * (sub + 1), :]
                                        .rearrange("p (h d) -> p h d", h=H))
                    return ktb, vb

                def div_write(h, j, qc, ysrc, rsrc):
                    recip = sm.tile([1, NCH], F32, name=f"rc{h}", tag=f"rc{h}")
                    nc.vector.reciprocal(out=recip, in_=rsrc)
                    rb = sm.tile([D, NCH], F32, name=f"rb{h}", tag=f"rb{h}")
                    nc.gpsimd.partition_broadcast(rb, recip)
                    nc.vector.tensor_tensor(out=yT[64 * h:64 * (h + 1), j, NCH * qc:NCH * (qc + 1)],
                                            in0=ysrc, in1=rb, op=ALU.mult)

                def proj(trange):
                    for t in trange:
                        for n in range(4):
                            ps = pr_ps.tile([P, NCH], F32, name="prps", tag="prps")
                            for k in range(4):
                                nc.tensor.matmul(ps[:, :], yT[:, 2 * k:2 * k + 2, P * t:P * (t + 1)],
                                                 wo_sb[:, 2 * k:2 * k + 2, NCH * n:NCH * (n + 1)],
                                                 start=(k == 0), stop=(k == 3), perf_mode=DR)
                            nc.vector.tensor_tensor(out=x2[:, t, NCH * n:NCH * (n + 1)], in0=ps[:, :],
                                                    in1=x_sb[:, t, NCH * n:NCH * (n + 1)], op=ALU.add)

                def score_block(j, s, kt, st, ktb, qcs):
                    """st: psum [P, 2h, len(qcs), NCH]. Emits scores + tri for kt."""
                    par = kt % 2
                    for h in range(2):
                        for ci, qc in enumerate(qcs):
                            sl = slot(s, kt, qc)
                            nc.tensor.matmul(
                                st[:, h, ci, :],
                                ktb[64 * h:64 * (h + 1), kt:kt + 2, j, :],
                                qz[64 * h:64 * (h + 1), :, j, NCH * qc:NCH * (qc + 1)],
                                start=True, stop=True, perf_mode=DR,
                                tile_position=(64 * h, 0))
                            # diagonal triangle: -256 into the masked region
                            nc.tensor.matmul(
                                st[:, h, ci, P * par:P * (par + 1)],
                                negI[:, :], mrhs[:, sl, :],
                                start=False, stop=True, skip_group_check=True)
                            if par == 1:
                                # odd diagonal block: first q-half fully dead
                                nc.tensor.matmul(
                                    st[:, h, ci, 0:P],
                                    negI[:, :], mkill[:, sl, :],
                                    start=False, stop=True, skip_group_check=True)

                def exp_block(j, s, kt, st, et, kslot, qcs):
                    for ci, qc in enumerate(qcs):
                        sl = slot(s, kt, qc)
                        nc.scalar.activation(
                            out=et[:, kslot, :, ci, :], in_=st[:, :, ci, :],
                            func=AF.Exp, scale=SCALE, bias=be[:, sl:sl + 1])

                def av_block(j, pair, et, vb, ya, cis, start, stops):
                    for h in range(2):
                        for ei, (ci, stop) in enumerate(zip(cis, stops)):
                            nc.tensor.matmul(
                                ya[h][:, ci, :],
                                vb[:, 2 * pair:2 * pair + 2, 2 * j + h, :],
                                et[:, :, h, ei, :],
                                start=start, stop=stop, perf_mode=DR)

                # ---- load both phases up front ----
                ktbA, vbA = load_kv(0)
                ktbB, vbB = load_kv(1)
                with tc.tile_pool(name="at_ps0", bufs=1, space="PSUM") as at_ps:
                    for j in range(8):
                        ya = [at_ps.tile([D + 1, 2, NCH], F32, name=f"ya{h}", tag=f"ya{h}", bufs=1)
                              for h in range(2)]
                        # ---- phase A: keys 0:1024, both chunks ----
                        for pair in range(4):
                            et = sm.tile([P, 2, 2, 2, NCH], FP8D, name="et", tag="et", bufs=2)
                            for kslot in range(2):
                                kt = 2 * pair + kslot
                                st = at_ps.tile([P, 2, 2, NCH], F32, name="st", tag="st",
                                                bufs=2)
                                score_block(j, 0, kt, st, ktbA, (0, 1))
                                exp_block(j, 0, kt, st, et, kslot, (0, 1))
                            av_block(j, pair, et, vbA, ya, (0, 1),
                                     start=(pair == 0),
                                     stops=(pair == 3, False))
                        for h in range(2):
                            div_write(h, j, 0, ya[h][0:D, 0, :], ya[h][D:D + 1, 0, :])
                        # ---- phase B: keys 1024:2048, chunk 1 only ----
                        for pair in range(4):
                            etb = sm.tile([P, 2, 2, 1, NCH], FP8D, name="etb", tag="etb", bufs=2)
                            for kslot in range(2):
                                kt = 2 * pair + kslot
                                stb = at_ps.tile([P, 2, 2, NCH], F32, name="st", tag="st",
                                                 bufs=2)
                                score_block(j, 1, kt, stb, ktbB, (1,))
                                exp_block(j, 1, kt, stb, etb, kslot, (1,))
                            av_block(j, pair, etb, vbB, ya, (1,),
                                     start=False, stops=(pair == 3,))
                        for h in range(2):
                            div_write(h, j, 1, ya[h][0:D, 1, :], ya[h][D:D + 1, 1, :])

                proj([0, 1, 2, 3])

        # ================= LN2 + MLP =================
        with tc.tile_pool(name="mlpp", bufs=1) as mp, \
             tc.tile_pool(name="wmlp", bufs=3) as wmp:
            ln2 = layer_norm(x2, mp, "ln2", dt=BF16)
            xln2T = transpose_to(ln2, mp, "xln2T", dt=FC1D, idn=ident_bf)
            for t in range(4):
                nc.vector.tensor_tensor(out=x2[:, t, :], in0=x2[:, t, :], in1=b2_bc, op=ALU.add)

            h_sb = mp.tile([P, 32, 512], FC2D)
            for half in range(2):
                with tc.tile_pool(name=f"mlp_ps{half}", bufs=1, space="PSUM") as mlp_ps:
                    ops = [mlp_ps.tile([P, 512], F32, name=f"ops{t}", tag=f"ops{t}", bufs=1)
                           for t in range(4)]
                    for m in range(32):
                        if half == 0:
                            if m % 4 == 0:
                                wfc = wmp.tile([P, 8, 512], FC1D, name="wfc", tag="wfc")
                                nc.sync.dma_start(out=wfc,
                                                    in_=wfc_d[:, 512 * (m // 4):512 * (m // 4 + 1)]
                                                    .rearrange("(kc kp) n -> kp kc n", kp=P))
                            mo = P * (m % 4)
                            fps = mlp_ps.tile([P, 512], F32, name="fps", tag="fps", bufs=4)
                            if FP8_FC1:
                                for k in range(4):
                                    for hh in range(2):
                                        nc.tensor.matmul(fps[:, 256 * hh:256 * (hh + 1)],
                                                         wfc[:, 2 * k:2 * k + 2, mo:mo + P],
                                                         xln2T[:, 2 * k:2 * k + 2, 256 * hh:256 * (hh + 1)],
                                                         start=(k == 0), stop=(k == 3), perf_mode=DR)
                            else:
                                for k in range(8):
                                    nc.tensor.matmul(fps[:, :], wfc[:, k, mo:mo + P], xln2T[:, k, :],
                                                     start=(k == 0), stop=(k == 7))
                            nc.scalar.activation(out=h_sb[:, m, :], in_=fps[:, :], func=AF.Gelu,
                                                 bias=bfc_sb[:, m:m + 1], scale=1.0)
                        if m % 4 == 0:
                            w2 = wmp.tile([P, 4, 512], FC2D, name="w2", tag="w2", bufs=3)
                            nc.scalar.dma_start(out=w2, in_=wfc2_d[P * m:P * (m + 4),
                                                               512 * half:512 * (half + 1)]
                                                .rearrange("(mc mp) n -> mp mc n", mp=P))
                        if FP8_FC2:
                            if m % 2 == 0:
                                for t in range(4):
                                    for n in range(2):
                                        nc.tensor.matmul(
                                            ops[t][:, 256 * n:256 * (n + 1)],
                                            h_sb[:, m:m + 2, P * t:P * (t + 1)],
                                            w2[:, (m % 4):(m % 4) + 2, 256 * n:256 * (n + 1)],
                                            start=(m == 0), stop=(m == 30), perf_mode=DR)
                        else:
                            for t in range(4):
                                nc.tensor.matmul(ops[t][:, :], h_sb[:, m, P * t:P * (t + 1)],
                                                 w2[:, m % 4, :], start=(m == 0), stop=(m == 31))
                    for t in range(4):
                        nc.vector.tensor_tensor(out=x2[:, t, 512 * half:512 * (half + 1)],
                                                in0=ops[t][:, :],
                                                in1=x2[:, t, 512 * half:512 * (half + 1)], op=ALU.add)
                        if half == 1:
                            nc.sync.dma_start(out=out_ext[P * t:P * (t + 1), :], in_=x2[:, t, :])

    nc.finalize()
    return nc


def _get_nc():
    if "nc" not in _CACHE:
        _CACHE["nc"] = _build()
    return _CACHE["nc"]


def _prep(**inputs):
    f = lambda a: np.asarray(a, dtype=np.float32)
    x = f(inputs["x"])
    ln1_g, ln1_b = f(inputs["ln1_g"]), f(inputs["ln1_b"])
    ln2_g, ln2_b = f(inputs["ln2_g"]), f(inputs["ln2_b"])
    W_attn, b_attn = f(inputs["W_attn"]), f(inputs["b_attn"])
    W_o, b_o = f(inputs["W_o"]), f(inputs["b_o"])
    W_fc, b_fc = f(inputs["W_fc"]), f(inputs["b_fc"])
    W_fc2, b_fc2 = f(inputs["W_fc2"]), f(inputs["b_fc2"])

    # fold LN affine params into the next matmul
    W_attn_e = ln1_g[:, None] * W_attn
    b_attn_e = b_attn + ln1_b @ W_attn
    W_fc_e = ln2_g[:, None] * W_fc
    b_fc_e = b_fc + ln2_b @ W_fc
    # V bias contributes a constant through attention: fold b_v @ W_o into
    # the residual bias (K bias is softmax-invariant and dropped).
    rb = b_o + b_attn_e[2 * C:3 * C] @ W_o

    fc1d = FP8 if FP8_FC1 else ml_dtypes.bfloat16
    fc2d = FP8 if FP8_FC2 else ml_dtypes.bfloat16

    in_maps = []
    for r in range(N_CORES):
        b, p = divmod(r, 4)
        c0, c1 = p, 7 - p
        xs = np.concatenate([x[b, NCH * c0:NCH * (c0 + 1)],
                             x[b, NCH * c1:NCH * (c1 + 1)]], axis=0)
        in_maps.append({
            "x": np.ascontiguousarray(xs),
            "qbase": np.array([[NCH * c0, NCH * c1]], dtype=np.float32),
            "wq": W_attn_e[:, 0:C].astype(FP8),
            "wk": W_attn_e[:, C:2 * C].astype(FP8),
            "wv": W_attn_e[:, 2 * C:3 * C].astype(FP8),
            "bq": b_attn_e[0:C],
            "wo": W_o.astype(FP8), "rb": rb,
            "w_fc": W_fc_e.astype(fc1d), "b_fc": b_fc_e,
            "w_fc2": W_fc2.astype(fc2d), "b_fc2": b_fc2,
        })

    def assemble(results):
        out = np.empty((B, T, C), dtype=np.float32)
        for r in range(N_CORES):
            b, p = divmod(r, 4)
            c0, c1 = p, 7 - p
            o = results[r]["out"]
            out[b, NCH * c0:NCH * (c0 + 1)] = o[0:NCH]
            out[b, NCH * c1:NCH * (c1 + 1)] = o[NCH:TOK]
        return out

    return in_maps, assemble


def kernel(**inputs):
    from concourse.bass_utils import run_bass_kernel_spmd

    in_maps, assemble = _prep(**inputs)
    res = run_bass_kernel_spmd(_get_nc(), in_maps, list(range(N_CORES)))
    return assemble(res.results)


# revision 15
# speedup vs baseline: 1.5109x; 1.0354x over previous
"""Transformer block (pre-LN causal MHA + GELU MLP) on 8 trn2 NeuronCores.

Sharding: core r handles batch b=r//4, group position p=r%4, owning token
chunks {p, 7-p} of eight 256-token chunks (causally balanced zigzag).
Sequence-parallel everywhere except attention: K^T and V for the full batch
are exchanged via fp8 AllGathers inside each 4-core batch group.

All heavy matmuls run in fp8e4 with DoubleRow perf mode (2 contraction
k-tiles per instruction at 0.5 cycles/row): QKV projections, attention
scores (K=64 with a zeroed second subtile on the Q side), attention*V
(key-tile pairs), output projection, and the MLP (precision tier
selectable per matmul via the FP8_* flags).

Masking is done on the PE + Act engines instead of element-wise DVE
multiplies: fully-masked (key-block, chunk) tiles get exp bias -30 from a
data-driven per-tile bias table (exp underflows to 0 in fp8), and the two
diagonal key-blocks per chunk get -256 added to the masked triangle via a
single extra matmul (lhsT=-256*I, rhs=triangle indicator built from qbase)
before the exp. Scores carry no 1/sqrt(d) or softmax-max handling: the
scale (0.125) and a -4*ln2 range shift are folded into the exp activation
(exp output ~ exp(s)/16 stays within fp8e4 range; the shift cancels in the
softmax division).

Bias handling: K bias is dropped (softmax is invariant to per-query score
shifts), V bias is folded into the residual bias on the host
(b_o + b_v @ W_o), Q bias is applied on the PSUM->SBUF copy, fc bias rides
the GELU activation, fc2 bias is pre-added to the residual.

LN rsqrt = exp(-0.5*ln(var+eps)) so LN1/attention/LN2 share one activation
table (natural_log_exp) and only the MLP's gelu forces a table switch.

Self-contained: hardcodes B=2, T=2048, C=1024, H=16, D=64, hidden=4096.
"""
import sys

if "/opt/trn_rl_repo" not in sys.path:
    sys.path.insert(0, "/opt/trn_rl_repo")

import numpy as np
import ml_dtypes

B, T, C, H = 2, 2048, 1024, 16
D = C // H            # 64
MH = 4 * C            # 4096 mlp hidden
EPS = 1e-5
P = 128
TOK = 512             # tokens per core
NCH = 256             # tokens per chunk
N_CORES = 8
EXPB = -2.7725887     # -4*ln2: exp emits exp(s)/16
SCALE = 0.125         # 1/sqrt(D)

# precision tiers (fp8 DoubleRow vs bf16) — tuned empirically
FP8_FC1 = True
FP8_FC2 = True

FP8 = ml_dtypes.float8_e4m3

_CACHE: dict = {}


def _build(mock_cc=False):
    import concourse.tile as tile
    from concourse import bacc, mybir
    from concourse.masks import make_identity
    from contextlib import ExitStack

    F32 = mybir.dt.float32
    BF16 = mybir.dt.bfloat16
    FP8D = mybir.dt.float8e4
    I32 = mybir.dt.int32
    AF = mybir.ActivationFunctionType
    ALU = mybir.AluOpType
    DR = mybir.MatmulPerfMode.DoubleRow

    FC1D = FP8D if FP8_FC1 else BF16
    FC2D = FP8D if FP8_FC2 else BF16

    nc = bacc.Bacc()

    # ---------------- I/O ----------------
    x_in = nc.declare_dram_parameter("x", [TOK, C], F32, isOutput=False)
    qbase_in = nc.declare_dram_parameter("qbase", [1, 2], F32, isOutput=False)
    wq_d = nc.declare_dram_parameter("wq", [C, C], FP8D, isOutput=False)
    wk_d = nc.declare_dram_parameter("wk", [C, C], FP8D, isOutput=False)
    wv_d = nc.declare_dram_parameter("wv", [C, C], FP8D, isOutput=False)
    bq_d = nc.declare_dram_parameter("bq", [C], F32, isOutput=False)
    wo_d = nc.declare_dram_parameter("wo", [C, C], FP8D, isOutput=False)
    rb_d = nc.declare_dram_parameter("rb", [C], F32, isOutput=False)
    wfc_d = nc.declare_dram_parameter("w_fc", [C, MH], FC1D, isOutput=False)
    bfc_d = nc.declare_dram_parameter("b_fc", [MH], F32, isOutput=False)
    wfc2_d = nc.declare_dram_parameter("w_fc2", [MH, C], FC2D, isOutput=False)
    bfc2_d = nc.declare_dram_parameter("b_fc2", [C], F32, isOutput=False)
    out_ext = nc.declare_dram_parameter("out", [TOK, C], F32, isOutput=True)

    # internal DRAM for the collectives (half s=0: keys 0:1024, s=1: 1024:2048)
    kt_in = [nc.dram_tensor(f"kt_in_{s}", [C, NCH], FP8D) for s in range(2)]
    v_in = [nc.dram_tensor(f"v_in_{s}", [NCH, C], FP8D) for s in range(2)]
    kt_all = [nc.dram_tensor(f"kt_all_{s}", [4 * C, NCH], FP8D) for s in range(2)]
    v_all = [nc.dram_tensor(f"v_all_{s}", [4 * NCH, C], FP8D) for s in range(2)]
    stash_d = nc.dram_tensor("stash_d", [16, D + 1, NCH], F32)
    RG = [[0, 1, 2, 3], [4, 5, 6, 7]]

    with tile.TileContext(nc) as tc, ExitStack() as ctx:
        # ---------- pools: outer (whole kernel) ----------
        const = ctx.enter_context(tc.tile_pool(name="const", bufs=1))
        outer = ctx.enter_context(tc.tile_pool(name="outer", bufs=1))
        sm = ctx.enter_context(tc.tile_pool(name="sm", bufs=2))

        # ---------- constants ----------
        ident = const.tile([P, P], F32)
        make_identity(nc, ident)
        ident_bf = const.tile([P, P], BF16)
        nc.vector.tensor_copy(out=ident_bf, in_=ident)
        eps_t = const.tile([P, 1], F32)
        nc.vector.memset(eps_t, EPS)
        ones128 = const.tile([P, P], F32)
        nc.vector.memset(ones128, 1.0)
        # -256 * I in fp8 (tri-mask stationary operand)
        negI = const.tile([P, P], FP8D)
        negI_f = const.tile([P, P], F32)
        nc.vector.tensor_scalar(out=negI_f, in0=ident, scalar1=-256.0, scalar2=None,
                                op0=ALU.mult)
        nc.vector.tensor_copy(out=negI, in_=negI_f)

        bq_sb = const.tile([P, 8], F32)     # q bias -> [128, 8]
        nc.sync.dma_start(out=bq_sb, in_=bq_d[0:C].rearrange("(f p) -> p f", p=P))
        bfc_sb = const.tile([P, 32], F32)
        nc.sync.dma_start(out=bfc_sb, in_=bfc_d[:].rearrange("(f p) -> p f", p=P))
        rb_bc = const.tile([P, C], F32)     # residual bias (b_o + b_v@W_o) bcast
        nc.sync.dma_start(out=rb_bc, in_=rb_d[:].rearrange("(a c) -> a c", a=1).to_broadcast((P, C)))
        b2_bc = const.tile([P, C], F32)
        nc.sync.dma_start(out=b2_bc, in_=bfc2_d[:].rearrange("(a c) -> a c", a=1).to_broadcast((P, C)))

        # qbase + iotas for mask tables
        qbase_sb = const.tile([1, 2], F32)
        nc.sync.dma_start(out=qbase_sb, in_=qbase_in[:, :])
        kidx_i = const.tile([P, 1], I32)
        nc.gpsimd.iota(kidx_i, pattern=[[0, 1]], base=0, channel_multiplier=1)
        kidx_f = const.tile([P, 1], F32)
        nc.vector.tensor_copy(out=kidx_f, in_=kidx_i)
        qio_i = const.tile([1, P], I32)
        nc.gpsimd.iota(qio_i, pattern=[[1, P]], base=0, channel_multiplier=0)
        qio_f = const.tile([1, P], F32)
        nc.vector.tensor_copy(out=qio_f, in_=qio_i)
        # TRI[k, q] = 1 if q < k else 0  (masked region of an aligned 128-diag)
        qio_bc = const.tile([P, P], F32)
        nc.gpsimd.partition_broadcast(qio_bc, qio_f)
        tri_f = const.tile([P, P], F32)
        nc.vector.tensor_scalar(out=tri_f, in0=qio_bc, scalar1=kidx_f, scalar2=None,
                                op0=ALU.is_lt)

        # ---- per-(phase, kt, chunk) exp bias table: alive -> EXPB, dead -> -30
        # slot order: (s, kt, c) -> 32 slots (s in 0..1, kt 0..7, c 0..1)
        kb_i = const.tile([1, 32], I32)
        nc.gpsimd.iota(kb_i, pattern=[[1024, 2], [128, 8], [0, 2]], base=0,
                       channel_multiplier=0)
        kb_f = const.tile([1, 32], F32)
        nc.vector.tensor_copy(out=kb_f, in_=kb_i)
        csel_i = const.tile([1, 32], I32)   # 0,1,0,1,... chunk selector
        nc.gpsimd.iota(csel_i, pattern=[[0, 2], [0, 8], [1, 2]], base=0,
                       channel_multiplier=0)
        csel_f = const.tile([1, 32], F32)
        nc.vector.tensor_copy(out=csel_f, in_=csel_i)
        # qb_slot = qbase[c0] + csel*(qbase[c1]-qbase[c0])
        qdiff = const.tile([1, 1], F32)
        nc.vector.tensor_scalar(out=qdiff, in0=qbase_sb[0:1, 1:2],
                                scalar1=qbase_sb[0:1, 0:1], scalar2=None,
                                op0=ALU.subtract)
        qb_slot = const.tile([1, 32], F32)
        nc.vector.tensor_scalar(out=qb_slot, in0=csel_f, scalar1=qdiff,
                                scalar2=qbase_sb[0:1, 0:1], op0=ALU.mult, op1=ALU.add)
        # alive = (qb_slot + 255 >= kb)  <=>  qb_slot - kb >= -255
        alive = const.tile([1, 32], F32)
        nc.vector.tensor_tensor(out=alive, in0=qb_slot, in1=kb_f, op=ALU.subtract)
        nc.vector.tensor_scalar(out=alive, in0=alive, scalar1=-255.0, scalar2=None,
                                op0=ALU.is_ge)
        be_row = const.tile([1, 32], F32)   # -30 + alive*(30+EXPB)
        nc.vector.tensor_scalar(out=be_row, in0=alive, scalar1=30.0 + EXPB,
                                scalar2=-30.0, op0=ALU.mult, op1=ALU.add)
        be = const.tile([P, 32], F32)
        nc.gpsimd.partition_broadcast(be, be_row)

        # ---- tri-mask rhs table: mrhs[:, slot, :] = TRI * diag(slot)
        # diag(slot) = 1 iff kb[slot] == qb_slot + 128*parity(kt)
        par_i = const.tile([1, 32], I32)
        nc.gpsimd.iota(par_i, pattern=[[0, 2], [0, 4], [128, 2], [0, 2]], base=0,
                       channel_multiplier=0)   # (s, ktpair, par, c) -> 128*(kt%2)
        par_f = const.tile([1, 32], F32)
        nc.vector.tensor_copy(out=par_f, in_=par_i)
        dfl = const.tile([1, 32], F32)
        nc.vector.tensor_tensor(out=dfl, in0=kb_f, in1=par_f, op=ALU.subtract)
        nc.vector.tensor_tensor(out=dfl, in0=dfl, in1=qb_slot, op=ALU.is_equal)
        dflb = const.tile([P, 32], F32)
        nc.gpsimd.partition_broadcast(dflb, dfl)
        mrhs = const.tile([P, 32, P], FP8D)
        for sl in range(32):
            nc.vector.tensor_scalar(out=mrhs[:, sl, :], in0=tri_f,
                                    scalar1=dflb[:, sl:sl + 1], scalar2=None,
                                    op0=ALU.mult)
        # full-kill pattern for the odd diagonal block's dead first q-half
        mkill = const.tile([P, 32, P], FP8D)
        for sl in range(32):
            if (sl // 2) % 2 == 1:   # odd kt slots only
                nc.vector.tensor_scalar(out=mkill[:, sl, :], in0=ones128,
                                        scalar1=dflb[:, sl:sl + 1], scalar2=None,
                                        op0=ALU.mult)

        def slot(s, kt, c):
            return s * 16 + kt * 2 + c

        # ---------- helpers ----------
        def layer_norm(src, dst_pool, tag, dt=BF16):
            # batched rsqrt: one Rsqrt call -> one act-table load per LN
            ln = dst_pool.tile([P, 4, C], dt, name=tag, tag=tag)
            mvs = sm.tile([P, 4, 2], F32, name=f"mvs{tag}", tag="lnmvs", bufs=1)
            for t in range(4):
                stats = sm.tile([P, 2, 6], F32, name="lnstats", tag="lnstats")
                nc.vector.bn_stats(out=stats[:, 0, :], in_=src[:, t, 0:512])
                nc.vector.bn_stats(out=stats[:, 1, :], in_=src[:, t, 512:1024])
                nc.vector.bn_aggr(out=mvs[:, t, :], in_=stats)
            rstd4 = sm.tile([P, 4], F32, name=f"rstd{tag}", tag="lnrstd", bufs=1)
            nc.scalar.activation(out=rstd4, in_=mvs[:, :, 1], func=AF.Sqrt,
                                 bias=eps_t, scale=1.0)
            nc.vector.reciprocal(out=rstd4, in_=rstd4)
            for t in range(4):
                nc.vector.tensor_scalar(out=ln[:, t, :], in0=src[:, t, :],
                                        scalar1=mvs[:, t, 0:1], scalar2=rstd4[:, t:t + 1],
                                        op0=ALU.subtract, op1=ALU.mult)
            return ln

        def transpose_to(lnt, dst_pool, dst_tag, dt, idn):
            xt = dst_pool.tile([P, 8, TOK], dt, name=dst_tag, tag=dst_tag)
            with tc.tile_pool(name="tp_ps", bufs=2, space="PSUM") as tp_ps:
                for t in range(4):
                    for f in range(8):
                        pt = tp_ps.tile([P, P], lnt.dtype, name="tpt", tag="tpt",
                                        padded_shape=[P, 2 * P])
                        nc.tensor.transpose(pt[:, :], lnt[:, t, P * f:P * (f + 1)], idn)
                        eng = nc.vector if f % 2 == 0 else nc.scalar
                        if f % 2 == 0:
                            nc.vector.tensor_copy(out=xt[:, f, P * t:P * (t + 1)], in_=pt[:, :])
                        else:
                            nc.scalar.activation(out=xt[:, f, P * t:P * (t + 1)], in_=pt[:, :],
                                                 func=AF.Copy)
            return xt

        x2 = outer.tile([P, 4, C], F32)
        yT = outer.tile([P, 8, TOK], FP8D)

        with tc.tile_pool(name="mid", bufs=1) as mid:
            x_sb = mid.tile([P, 4, C], F32)
            for t in range(4):
                nc.sync.dma_start(out=x_sb[:, t, :], in_=x_in[P * t:P * (t + 1), :])
            # qz: [part, sub(2), j, tok]  sub1 = zeros (DoubleRow zero-subtile)
            qz = mid.tile([P, 2, 8, TOK], FP8D)
            nc.gpsimd.memset(qz[:, 1, :, :], 0.0)

            # ================= qkv =================
            with tc.tile_pool(name="qkvp", bufs=1) as qp, \
                 tc.tile_pool(name="wqkv", bufs=2) as wp:
                ln1 = layer_norm(x_sb, qp, "ln")
                xT8 = transpose_to(ln1, qp, "xT8", FP8D, ident_bf)
                qkv_ps_cm = tc.tile_pool(name="qkv_ps", bufs=3, space="PSUM")
                qkv_ps = qkv_ps_cm.__enter__()

                wk_sb = wp.tile([P, 8, C], FP8D, name="wk", tag="wk")
                nc.scalar.dma_start(out=wk_sb, in_=wk_d[:, :].rearrange("(kc kp) n -> kp kc n", kp=P))
                # K^T feature tiles -> kt_in halves (K bias dropped: softmax-invariant)
                for f in range(8):
                    fo = P * f
                    ps = qkv_ps.tile([P, TOK], F32, name="kps", tag="qkvps")
                    for k in range(4):
                        for hh in range(2):
                            nc.tensor.matmul(ps[:, TOK // 2 * hh:TOK // 2 * (hh + 1)],
                                             wk_sb[:, 2 * k:2 * k + 2, fo:fo + P],
                                             xT8[:, 2 * k:2 * k + 2, 256 * hh:256 * (hh + 1)],
                                             start=(k == 0 and hh == 0),
                                             stop=(k == 3 and hh == 1), perf_mode=DR)
                    kt_sb = sm.tile([P, TOK], FP8D, name="kt_sb", tag="kt_sb", bufs=2)
                    nc.scalar.activation(out=kt_sb, in_=ps[:, :], func=AF.Copy)
                    for s in range(2):
                        nc.sync.dma_start(out=kt_in[s][P * f:P * (f + 1), :],
                                          in_=kt_sb[:, NCH * s:NCH * (s + 1)])
                # kt gathers fire as soon as K^T is written
                for s in range(2):
                    if mock_cc:
                        nc.gpsimd.dma_start(out=kt_all[s][0:C, :], in_=kt_in[s][:, :])
                    else:
                        nc.gpsimd.collective_compute("AllGather", ALU.bypass,
                                                     ins=[kt_in[s][:, :]], outs=[kt_all[s][:, :]],
                                                     replica_groups=RG)
                # V token tiles -> v_in halves (V bias folded into residual bias)
                wv_sb = wp.tile([P, 8, C], FP8D, name="wv", tag="wk")
                nc.sync.dma_start(out=wv_sb, in_=wv_d[:, :].rearrange("(kc kp) n -> kp kc n", kp=P))
                for t in range(4):
                    ps = qkv_ps.tile([P, C], F32, name="vps", tag="vps", bufs=2)
                    for k in range(4):
                        for n in range(4):
                            nc.tensor.matmul(ps[:, NCH * n:NCH * (n + 1)],
                                             xT8[:, 2 * k:2 * k + 2, P * t:P * (t + 1)],
                                             wv_sb[:, 2 * k:2 * k + 2, NCH * n:NCH * (n + 1)],
                                             start=(k == 0 and n % 2 == 0),
                                             stop=(k == 3 and n % 2 == 1), perf_mode=DR)
                    v_sb = sm.tile([P, C], FP8D, name="v_sb", tag="v_sb")
                    nc.scalar.activation(out=v_sb, in_=ps[:, :], func=AF.Copy)
                    sh, row = divmod(t, 2)
                    nc.sync.dma_start(out=v_in[sh][P * row:P * (row + 1), :], in_=v_sb)
                    if row == 1:
                        if mock_cc:
                            nc.gpsimd.dma_start(out=v_all[sh][0:NCH, :], in_=v_in[sh][:, :])
                        else:
                            nc.gpsimd.collective_compute("AllGather", ALU.bypass,
                                                         ins=[v_in[sh][:, :]], outs=[v_all[sh][:, :]],
                                                         replica_groups=RG)

                # Q^T feature tiles (stay local); bias on copy, scale folded in exp
                wq_sb = wp.tile([P, 8, C], FP8D, name="wq", tag="wk")
                nc.sync.dma_start(out=wq_sb, in_=wq_d[:, :].rearrange("(kc kp) n -> kp kc n", kp=P))
                for f in range(8):
                    fo = P * f
                    ps = qkv_ps.tile([P, TOK], F32, name="qps", tag="qkvps")
                    for k in range(4):
                        for hh in range(2):
                            nc.tensor.matmul(ps[:, TOK // 2 * hh:TOK // 2 * (hh + 1)],
                                             wq_sb[:, 2 * k:2 * k + 2, fo:fo + P],
                                             xT8[:, 2 * k:2 * k + 2, 256 * hh:256 * (hh + 1)],
                                             start=(k == 0 and hh == 0),
                                             stop=(k == 3 and hh == 1), perf_mode=DR)
                    nc.scalar.activation(out=qz[:, 0, f, :], in_=ps[:, :], func=AF.Identity,
                                         bias=bq_sb[:, f:f + 1], scale=1.0)
                qkv_ps_cm.__exit__(None, None, None)

            # ============ attention (+ proj overlapped into phase B) ============
            with tc.tile_pool(name="attp", bufs=1) as ap, \
                 tc.tile_pool(name="projp", bufs=1) as pp, \
                 tc.tile_pool(name="pr_ps", bufs=2, space="PSUM") as pr_ps:
                wo_sb = pp.tile([P, 8, C], FP8D)
                nc.sync.dma_start(out=wo_sb, in_=wo_d[:, :].rearrange("(kc kp) n -> kp kc n", kp=P))
                for t in range(4):
                    nc.gpsimd.tensor_tensor(out=x_sb[:, t, :], in0=x_sb[:, t, :], in1=rb_bc, op=ALU.add)

                def load_kv(s):
                    # ktb: [part(2h d), kt-slot(8+1 pad), j, keys]
                    ktb = ap.tile([P, 9, 8, P], FP8D, name="ktb", tag="ktb", bufs=2)
                    nc.gpsimd.memset(ktb[:, 8, :, :], 0.0)
                    # vb: [part(key), kt-slot, hh, D+1]
                    vb = ap.tile([P, 8, 16, D + 1], FP8D, name="vb", tag="vb", bufs=2)
                    nc.vector.tensor_copy(out=vb[:, :, :, D:D + 1],
                                          in_=ones128.rearrange("p (a b) -> p a b", a=8)[:, :, 0:16])
                    for r in range(4):
                        nc.sync.dma_start(
                            out=ktb[:, 2 * r:2 * r + 2, :, :],
                            in_=kt_all[s][C * r:C * (r + 1), :].rearrange(
                                "(j p) (kb kc) -> p kb j kc", p=P, kb=2))
                        for sub in range(2):
                            nc.sync.dma_start(
                                out=vb[:, 2 * r + sub, :, 0:D],
                                in_=v_all[s][NCH * r + P * sub:NCH * r + P * (sub + 1), :]
                                        .rearrange("p (h d) -> p h d", h=H))
                    return ktb, vb

                def div_write(h, j, qc, ysrc, rsrc):
                    recip = sm.tile([1, NCH], F32, name=f"rc{h}", tag=f"rc{h}")
                    nc.vector.reciprocal(out=recip, in_=rsrc)
                    rb = sm.tile([D, NCH], F32, name=f"rb{h}", tag=f"rb{h}")
                    nc.gpsimd.partition_broadcast(rb, recip)
                    nc.vector.tensor_tensor(out=yT[64 * h:64 * (h + 1), j, NCH * qc:NCH * (qc + 1)],
                                            in0=ysrc, in1=rb, op=ALU.mult)

                def proj(trange):
                    for t in trange:
                        for nn in range(2):
                            ps = pr_ps.tile([P, 512], F32, name="prps", tag="prps")
                            for k in range(4):
                                for n2 in range(2):
                                    nc.tensor.matmul(
                                        ps[:, NCH * n2:NCH * (n2 + 1)],
                                        yT[:, 2 * k:2 * k + 2, P * t:P * (t + 1)],
                                        wo_sb[:, 2 * k:2 * k + 2, 512 * nn + NCH * n2:512 * nn + NCH * (n2 + 1)],
                                        start=(k == 0 and n2 == 0),
                                        stop=(k == 3 and n2 == 1), perf_mode=DR)
                            nc.vector.tensor_tensor(out=x2[:, t, 512 * nn:512 * (nn + 1)], in0=ps[:, :],
                                                    in1=x_sb[:, t, 512 * nn:512 * (nn + 1)], op=ALU.add)

                def score_block(j, s, kt, st, ktb, qcs):
                    """st: psum [P, 2h, len(qcs), NCH]. Emits scores + tri for kt."""
                    par = kt % 2
                    for h in range(2):
                        for ci, qc in enumerate(qcs):
                            sl = slot(s, kt, qc)
                            nc.tensor.matmul(
                                st[:, h, ci, :],
                                ktb[64 * h:64 * (h + 1), kt:kt + 2, j, :],
                                qz[64 * h:64 * (h + 1), :, j, NCH * qc:NCH * (qc + 1)],
                                start=(ci == 0), stop=(ci == len(qcs) - 1),
                                perf_mode=DR, tile_position=(64 * h, 0))
                            # diagonal triangle: -256 into the masked region
                            nc.tensor.matmul(
                                st[:, h, ci, P * par:P * (par + 1)],
                                negI[:, :], mrhs[:, sl, :],
                                start=False, stop=False, skip_group_check=True)
                            if par == 1:
                                # odd diagonal block: first q-half fully dead
                                nc.tensor.matmul(
                                    st[:, h, ci, 0:P],
                                    negI[:, :], mkill[:, sl, :],
                                    start=False, stop=False, skip_group_check=True)

                def exp_block(j, s, kt, st, et, kslot, qcs):
                    for ci, qc in enumerate(qcs):
                        sl = slot(s, kt, qc)
                        nc.scalar.activation(
                            out=et[:, kslot, :, ci, :], in_=st[:, :, ci, :],
                            func=AF.Exp, scale=SCALE, bias=be[:, sl:sl + 1])

                def av_block(j, pair, et, vb, ya, cis, starts, stops):
                    for h in range(2):
                        for ei, (ci, start, stop) in enumerate(zip(cis, starts, stops)):
                            nc.tensor.matmul(
                                ya[h][:, ci, :],
                                vb[:, 2 * pair:2 * pair + 2, 2 * j + h, :],
                                et[:, :, h, ei, :],
                                start=start, stop=stop, perf_mode=DR)

                # ---- load both phases up front ----
                ktbA, vbA = load_kv(0)
                ktbB, vbB = load_kv(1)
                with tc.tile_pool(name="at_ps0", bufs=1, space="PSUM") as at_ps:
                    for j in range(8):
                        ya = [at_ps.tile([D + 1, 2, NCH], F32, name=f"ya{h}", tag=f"ya{h}", bufs=1)
                              for h in range(2)]
                        # ---- phase A: keys 0:1024, both chunks ----
                        for pair in range(4):
                            et = sm.tile([P, 2, 2, 2, NCH], FP8D, name="et", tag="et", bufs=2)
                            for kslot in range(2):
                                kt = 2 * pair + kslot
                                st = at_ps.tile([P, 2, 2, NCH], F32, name="st", tag="st",
                                                bufs=2)
                                score_block(j, 0, kt, st, ktbA, (0, 1))
                                exp_block(j, 0, kt, st, et, kslot, (0, 1))
                            av_block(j, pair, et, vbA, ya, (0, 1),
                                     starts=(pair == 0, False),
                                     stops=(False, False))
                        for h in range(2):
                            div_write(h, j, 0, ya[h][0:D, 0, :], ya[h][D:D + 1, 0, :])
                        # ---- phase B: keys 1024:2048, chunk 1 only ----
                        for pair in range(4):
                            etb = sm.tile([P, 2, 2, 1, NCH], FP8D, name="etb", tag="etb", bufs=2)
                            for kslot in range(2):
                                kt = 2 * pair + kslot
                                stb = at_ps.tile([P, 2, 2, NCH], F32, name="st", tag="st",
                                                 bufs=2)
                                score_block(j, 1, kt, stb, ktbB, (1,))
                                exp_block(j, 1, kt, stb, etb, kslot, (1,))
                            av_block(j, pair, etb, vbB, ya, (1,),
                                     starts=(False,), stops=(pair == 3,))
                        for h in range(2):
                            div_write(h, j, 1, ya[h][0:D, 1, :], ya[h][D:D + 1, 1, :])

                proj([0, 1, 2, 3])

        # ================= LN2 + MLP =================
        with tc.tile_pool(name="mlpp", bufs=1) as mp, \
             tc.tile_pool(name="wmlp", bufs=3) as wmp:
            ln2 = layer_norm(x2, mp, "ln2", dt=BF16)
            xln2T = transpose_to(ln2, mp, "xln2T", dt=FC1D, idn=ident_bf)
            for t in range(4):
                nc.gpsimd.tensor_tensor(out=x2[:, t, :], in0=x2[:, t, :], in1=b2_bc, op=ALU.add)

            h_sb = mp.tile([P, 32, 512], FC2D)
            for half in range(2):
                with tc.tile_pool(name=f"mlp_ps{half}", bufs=1, space="PSUM") as mlp_ps:
                    ops = [mlp_ps.tile([P, 512], F32, name=f"ops{t}", tag=f"ops{t}", bufs=1)
                           for t in range(4)]
                    for m in range(32):
                        if half == 0:
                            if m % 4 == 0:
                                wfc = wmp.tile([P, 8, 512], FC1D, name="wfc", tag="wfc")
                                nc.sync.dma_start(out=wfc,
                                                    in_=wfc_d[:, 512 * (m // 4):512 * (m // 4 + 1)]
                                                    .rearrange("(kc kp) n -> kp kc n", kp=P))
                            mo = P * (m % 4)
                            fps = mlp_ps.tile([P, 512], F32, name="fps", tag="fps", bufs=4)
                            if FP8_FC1:
                                for k in range(4):
                                    for hh in range(2):
                                        nc.tensor.matmul(fps[:, 256 * hh:256 * (hh + 1)],
                                                         wfc[:, 2 * k:2 * k + 2, mo:mo + P],
                                                         xln2T[:, 2 * k:2 * k + 2, 256 * hh:256 * (hh + 1)],
                                                         start=(k == 0 and hh == 0),
                                                         stop=(k == 3 and hh == 1), perf_mode=DR)
                            else:
                                for k in range(8):
                                    nc.tensor.matmul(fps[:, :], wfc[:, k, mo:mo + P], xln2T[:, k, :],
                                                     start=(k == 0), stop=(k == 7))
                            nc.scalar.activation(out=h_sb[:, m, :], in_=fps[:, :], func=AF.Gelu,
                                                 bias=bfc_sb[:, m:m + 1], scale=1.0)
                        if m % 4 == 0:
                            w2 = wmp.tile([P, 4, 512], FC2D, name="w2", tag="w2", bufs=3)
                            nc.sync.dma_start(out=w2, in_=wfc2_d[P * m:P * (m + 4),
                                                               512 * half:512 * (half + 1)]
                                                .rearrange("(mc mp) n -> mp mc n", mp=P))
                        if FP8_FC2:
                            if m % 2 == 0:
                                for t in range(4):
                                    for n in range(2):
                                        nc.tensor.matmul(
                                            ops[t][:, 256 * n:256 * (n + 1)],
                                            h_sb[:, m:m + 2, P * t:P * (t + 1)],
                                            w2[:, (m % 4):(m % 4) + 2, 256 * n:256 * (n + 1)],
                                            start=(m == 0 and n == 0),
                                            stop=(m == 30 and n == 1), perf_mode=DR)
                        else:
                            for t in range(4):
                                nc.tensor.matmul(ops[t][:, :], h_sb[:, m, P * t:P * (t + 1)],
                                                 w2[:, m % 4, :], start=(m == 0), stop=(m == 31))
                    for t in range(4):
                        nc.vector.tensor_tensor(out=x2[:, t, 512 * half:512 * (half + 1)],
                                                in0=ops[t][:, :],
                                                in1=x2[:, t, 512 * half:512 * (half + 1)], op=ALU.add)
                        if half == 1:
                            nc.sync.dma_start(out=out_ext[P * t:P * (t + 1), :], in_=x2[:, t, :])

    nc.finalize()
    return nc


def _get_nc():
    if "nc" not in _CACHE:
        _CACHE["nc"] = _build()
    return _CACHE["nc"]


def _prep(**inputs):
    f = lambda a: np.asarray(a, dtype=np.float32)
    x = f(inputs["x"])
    ln1_g, ln1_b = f(inputs["ln1_g"]), f(inputs["ln1_b"])
    ln2_g, ln2_b = f(inputs["ln2_g"]), f(inputs["ln2_b"])
    W_attn, b_attn = f(inputs["W_attn"]), f(inputs["b_attn"])
    W_o, b_o = f(inputs["W_o"]), f(inputs["b_o"])
    W_fc, b_fc = f(inputs["W_fc"]), f(inputs["b_fc"])
    W_fc2, b_fc2 = f(inputs["W_fc2"]), f(inputs["b_fc2"])

    # fold LN affine params into the next matmul
    W_attn_e = ln1_g[:, None] * W_attn
    b_attn_e = b_attn + ln1_b @ W_attn
    W_fc_e = ln2_g[:, None] * W_fc
    b_fc_e = b_fc + ln2_b @ W_fc
    # V bias contributes a constant through attention: fold b_v @ W_o into
    # the residual bias (K bias is softmax-invariant and dropped).
    rb = b_o + b_attn_e[2 * C:3 * C] @ W_o

    fc1d = FP8 if FP8_FC1 else ml_dtypes.bfloat16
    fc2d = FP8 if FP8_FC2 else ml_dtypes.bfloat16

    in_maps = []
    for r in range(N_CORES):
        b, p = divmod(r, 4)
        c0, c1 = p, 7 - p
        xs = np.concatenate([x[b, NCH * c0:NCH * (c0 + 1)],
                             x[b, NCH * c1:NCH * (c1 + 1)]], axis=0)
        in_maps.append({
            "x": np.ascontiguousarray(xs),
            "qbase": np.array([[NCH * c0, NCH * c1]], dtype=np.float32),
            "wq": W_attn_e[:, 0:C].astype(FP8),
            "wk": W_attn_e[:, C:2 * C].astype(FP8),
            "wv": W_attn_e[:, 2 * C:3 * C].astype(FP8),
            "bq": b_attn_e[0:C],
            "wo": W_o.astype(FP8), "rb": rb,
            "w_fc": W_fc_e.astype(fc1d), "b_fc": b_fc_e,
            "w_fc2": W_fc2.astype(fc2d), "b_fc2": b_fc2,
        })

    def assemble(results):
        out = np.empty((B, T, C), dtype=np.float32)
        for r in range(N_CORES):
            b, p = divmod(r, 4)
            c0, c1 = p, 7 - p
            o = results[r]["out"]
            out[b, NCH * c0:NCH * (c0 + 1)] = o[0:NCH]
            out[b, NCH * c1:NCH * (c1 + 1)] = o[NCH:TOK]
        return out

    return in_maps, assemble


def kernel(**inputs):
    from concourse.bass_utils import run_bass_kernel_spmd

    in_maps, assemble = _prep(**inputs)
    res = run_bass_kernel_spmd(_get_nc(), in_maps, list(range(N_CORES)))
    return assemble(res.results)


# revision 17
# speedup vs baseline: 1.5182x; 1.0048x over previous
"""Transformer block (pre-LN causal MHA + GELU MLP) on 8 trn2 NeuronCores.

Sharding: core r handles batch b=r//4, group position p=r%4, owning token
chunks {p, 7-p} of eight 256-token chunks (causally balanced zigzag).
Sequence-parallel everywhere except attention: K^T and V for the full batch
are exchanged via fp8 AllGathers inside each 4-core batch group.

All heavy matmuls run in fp8e4 with DoubleRow perf mode (2 contraction
k-tiles per instruction at 0.5 cycles/row): QKV projections, attention
scores (K=64 with a zeroed second subtile on the Q side), attention*V
(key-tile pairs), output projection, and the MLP (precision tier
selectable per matmul via the FP8_* flags).

Masking is done on the PE + Act engines instead of element-wise DVE
multiplies: fully-masked (key-block, chunk) tiles get exp bias -30 from a
data-driven per-tile bias table (exp underflows to 0 in fp8), and the two
diagonal key-blocks per chunk get -256 added to the masked triangle via a
single extra matmul (lhsT=-256*I, rhs=triangle indicator built from qbase)
before the exp. Scores carry no 1/sqrt(d) or softmax-max handling: the
scale (0.125) and a -4*ln2 range shift are folded into the exp activation
(exp output ~ exp(s)/16 stays within fp8e4 range; the shift cancels in the
softmax division).

Bias handling: K bias is dropped (softmax is invariant to per-query score
shifts), V bias is folded into the residual bias on the host
(b_o + b_v @ W_o), Q bias is applied on the PSUM->SBUF copy, fc bias rides
the GELU activation, fc2 bias is pre-added to the residual.

LN rsqrt = exp(-0.5*ln(var+eps)) so LN1/attention/LN2 share one activation
table (natural_log_exp) and only the MLP's gelu forces a table switch.

Self-contained: hardcodes B=2, T=2048, C=1024, H=16, D=64, hidden=4096.
"""
import sys

if "/opt/trn_rl_repo" not in sys.path:
    sys.path.insert(0, "/opt/trn_rl_repo")

import numpy as np
import ml_dtypes

B, T, C, H = 2, 2048, 1024, 16
D = C // H            # 64
MH = 4 * C            # 4096 mlp hidden
EPS = 1e-5
P = 128
TOK = 512             # tokens per core
NCH = 256             # tokens per chunk
N_CORES = 8
EXPB = -2.7725887     # -4*ln2: exp emits exp(s)/16
SCALE = 0.125         # 1/sqrt(D)

# precision tiers (fp8 DoubleRow vs bf16) — tuned empirically
FP8_FC1 = True
FP8_FC2 = True

FP8 = ml_dtypes.float8_e4m3

_CACHE: dict = {}


def _build(mock_cc=False):
    import concourse.tile as tile
    from concourse import bacc, mybir
    from concourse.masks import make_identity
    from contextlib import ExitStack

    F32 = mybir.dt.float32
    BF16 = mybir.dt.bfloat16
    FP8D = mybir.dt.float8e4
    I32 = mybir.dt.int32
    AF = mybir.ActivationFunctionType
    ALU = mybir.AluOpType
    DR = mybir.MatmulPerfMode.DoubleRow

    FC1D = FP8D if FP8_FC1 else BF16
    FC2D = FP8D if FP8_FC2 else BF16

    nc = bacc.Bacc()

    # ---------------- I/O ----------------
    x_in = nc.declare_dram_parameter("x", [TOK, C], F32, isOutput=False)
    qbase_in = nc.declare_dram_parameter("qbase", [1, 2], F32, isOutput=False)
    wq_d = nc.declare_dram_parameter("wq", [C, C], FP8D, isOutput=False)
    wk_d = nc.declare_dram_parameter("wk", [C, C], FP8D, isOutput=False)
    wv_d = nc.declare_dram_parameter("wv", [C, C], FP8D, isOutput=False)
    bq_d = nc.declare_dram_parameter("bq", [C], F32, isOutput=False)
    wo_d = nc.declare_dram_parameter("wo", [C, C], FP8D, isOutput=False)
    rb_d = nc.declare_dram_parameter("rb", [C], F32, isOutput=False)
    wfc_d = nc.declare_dram_parameter("w_fc", [C, MH], FC1D, isOutput=False)
    bfc_d = nc.declare_dram_parameter("b_fc", [MH], F32, isOutput=False)
    wfc2_d = nc.declare_dram_parameter("w_fc2", [MH, C], FC2D, isOutput=False)
    bfc2_d = nc.declare_dram_parameter("b_fc2", [C], F32, isOutput=False)
    out_ext = nc.declare_dram_parameter("out", [TOK, C], F32, isOutput=True)

    # internal DRAM for the collectives (half s=0: keys 0:1024, s=1: 1024:2048)
    kt_in = [nc.dram_tensor(f"kt_in_{s}", [C, NCH], FP8D) for s in range(2)]
    v_in = [nc.dram_tensor(f"v_in_{s}", [NCH, C], FP8D) for s in range(2)]
    kt_all = [nc.dram_tensor(f"kt_all_{s}", [4 * C, NCH], FP8D) for s in range(2)]
    v_all = [nc.dram_tensor(f"v_all_{s}", [4 * NCH, C], FP8D) for s in range(2)]
    stash_d = nc.dram_tensor("stash_d", [16, D + 1, NCH], F32)
    RG = [[0, 1, 2, 3], [4, 5, 6, 7]]

    with tile.TileContext(nc) as tc, ExitStack() as ctx:
        # ---------- pools: outer (whole kernel) ----------
        const = ctx.enter_context(tc.tile_pool(name="const", bufs=1))
        outer = ctx.enter_context(tc.tile_pool(name="outer", bufs=1))
        sm = ctx.enter_context(tc.tile_pool(name="sm", bufs=2))

        # ---------- constants ----------
        ident = const.tile([P, P], F32)
        make_identity(nc, ident)
        ident_bf = const.tile([P, P], BF16)
        nc.vector.tensor_copy(out=ident_bf, in_=ident)
        eps_t = const.tile([P, 1], F32)
        nc.vector.memset(eps_t, EPS)
        ones128 = const.tile([P, P], F32)
        nc.vector.memset(ones128, 1.0)
        # -128 * I in fp8 (tri-mask stationary operand; exp(s/8-16) -> 0 in fp8)
        negI = const.tile([P, P], FP8D)
        negI_f = const.tile([P, P], F32)
        nc.vector.tensor_scalar(out=negI_f, in0=ident, scalar1=-128.0, scalar2=None,
                                op0=ALU.mult)
        nc.vector.tensor_copy(out=negI, in_=negI_f)

        bq_sb = const.tile([P, 8], F32)     # q bias -> [128, 8]
        nc.sync.dma_start(out=bq_sb, in_=bq_d[0:C].rearrange("(f p) -> p f", p=P))
        bfc_sb = const.tile([P, 32], F32)
        nc.sync.dma_start(out=bfc_sb, in_=bfc_d[:].rearrange("(f p) -> p f", p=P))
        rb_bc = const.tile([P, C], F32)     # residual bias (b_o + b_v@W_o) bcast
        nc.sync.dma_start(out=rb_bc, in_=rb_d[:].rearrange("(a c) -> a c", a=1).to_broadcast((P, C)))
        b2_bc = const.tile([P, C], F32)
        nc.sync.dma_start(out=b2_bc, in_=bfc2_d[:].rearrange("(a c) -> a c", a=1).to_broadcast((P, C)))

        # qbase + iotas for mask tables
        qbase_sb = const.tile([1, 2], F32)
        nc.sync.dma_start(out=qbase_sb, in_=qbase_in[:, :])
        kidx_i = const.tile([P, 1], I32)
        nc.gpsimd.iota(kidx_i, pattern=[[0, 1]], base=0, channel_multiplier=1)
        kidx_f = const.tile([P, 1], F32)
        nc.vector.tensor_copy(out=kidx_f, in_=kidx_i)
        qio_i = const.tile([1, P], I32)
        nc.gpsimd.iota(qio_i, pattern=[[1, P]], base=0, channel_multiplier=0)
        qio_f = const.tile([1, P], F32)
        nc.vector.tensor_copy(out=qio_f, in_=qio_i)
        # TRI[k, q] = 1 if q < k else 0  (masked region of an aligned 128-diag)
        qio_bc = const.tile([P, P], F32)
        nc.gpsimd.partition_broadcast(qio_bc, qio_f)
        tri_f = const.tile([P, P], F32)
        nc.vector.tensor_scalar(out=tri_f, in0=qio_bc, scalar1=kidx_f, scalar2=None,
                                op0=ALU.is_lt)

        # ---- per-(phase, kt, chunk) exp bias table: alive -> EXPB, dead -> -30
        # slot order: (s, kt, c) -> 32 slots (s in 0..1, kt 0..7, c 0..1)
        kb_i = const.tile([1, 32], I32)
        nc.gpsimd.iota(kb_i, pattern=[[1024, 2], [128, 8], [0, 2]], base=0,
                       channel_multiplier=0)
        kb_f = const.tile([1, 32], F32)
        nc.vector.tensor_copy(out=kb_f, in_=kb_i)
        csel_i = const.tile([1, 32], I32)   # 0,1,0,1,... chunk selector
        nc.gpsimd.iota(csel_i, pattern=[[0, 2], [0, 8], [1, 2]], base=0,
                       channel_multiplier=0)
        csel_f = const.tile([1, 32], F32)
        nc.vector.tensor_copy(out=csel_f, in_=csel_i)
        # qb_slot = qbase[c0] + csel*(qbase[c1]-qbase[c0])
        qdiff = const.tile([1, 1], F32)
        nc.vector.tensor_scalar(out=qdiff, in0=qbase_sb[0:1, 1:2],
                                scalar1=qbase_sb[0:1, 0:1], scalar2=None,
                                op0=ALU.subtract)
        qb_slot = const.tile([1, 32], F32)
        nc.vector.tensor_scalar(out=qb_slot, in0=csel_f, scalar1=qdiff,
                                scalar2=qbase_sb[0:1, 0:1], op0=ALU.mult, op1=ALU.add)
        # alive = (qb_slot + 255 >= kb)  <=>  qb_slot - kb >= -255
        alive = const.tile([1, 32], F32)
        nc.vector.tensor_tensor(out=alive, in0=qb_slot, in1=kb_f, op=ALU.subtract)
        nc.vector.tensor_scalar(out=alive, in0=alive, scalar1=-255.0, scalar2=None,
                                op0=ALU.is_ge)
        be_row = const.tile([1, 32], F32)   # -30 + alive*(30+EXPB)
        nc.vector.tensor_scalar(out=be_row, in0=alive, scalar1=30.0 + EXPB,
                                scalar2=-30.0, op0=ALU.mult, op1=ALU.add)
        be = const.tile([P, 32], F32)
        nc.gpsimd.partition_broadcast(be, be_row)

        # ---- tri-mask rhs table: mrhs[:, slot, :] = TRI * diag(slot)
        # diag(slot) = 1 iff kb[slot] == qb_slot + 128*parity(kt)
        par_i = const.tile([1, 32], I32)
        nc.gpsimd.iota(par_i, pattern=[[0, 2], [0, 4], [128, 2], [0, 2]], base=0,
                       channel_multiplier=0)   # (s, ktpair, par, c) -> 128*(kt%2)
        par_f = const.tile([1, 32], F32)
        nc.vector.tensor_copy(out=par_f, in_=par_i)
        dfl = const.tile([1, 32], F32)
        nc.vector.tensor_tensor(out=dfl, in0=kb_f, in1=par_f, op=ALU.subtract)
        nc.vector.tensor_tensor(out=dfl, in0=dfl, in1=qb_slot, op=ALU.is_equal)
        dflb = const.tile([P, 32], F32)
        nc.gpsimd.partition_broadcast(dflb, dfl)
        mrhs = const.tile([P, 32, P], FP8D)
        for sl in range(32):
            nc.vector.tensor_scalar(out=mrhs[:, sl, :], in0=tri_f,
                                    scalar1=dflb[:, sl:sl + 1], scalar2=None,
                                    op0=ALU.mult)
        # full-kill pattern for the odd diagonal block's dead first q-half
        mkill = const.tile([P, 32, P], FP8D)
        for sl in range(32):
            if (sl // 2) % 2 == 1:   # odd kt slots only
                nc.vector.tensor_scalar(out=mkill[:, sl, :], in0=ones128,
                                        scalar1=dflb[:, sl:sl + 1], scalar2=None,
                                        op0=ALU.mult)

        def slot(s, kt, c):
            return s * 16 + kt * 2 + c

        # ---------- helpers ----------
        def layer_norm(src, dst_pool, tag, dt=BF16):
            # per-t sqrt (Sqrt table shared across all 4 calls; Exp/Gelu load later)
            ln = dst_pool.tile([P, 4, C], dt, name=tag, tag=tag)
            for t in range(4):
                stats = sm.tile([P, 2, 6], F32, name="lnstats", tag="lnstats")
                nc.vector.bn_stats(out=stats[:, 0, :], in_=src[:, t, 0:512])
                nc.vector.bn_stats(out=stats[:, 1, :], in_=src[:, t, 512:1024])
                mv = sm.tile([P, 2], F32, name="lnmv", tag="lnmv")
                nc.vector.bn_aggr(out=mv, in_=stats)
                rstd = sm.tile([P, 1], F32, name="lnrstd", tag="lnrstd")
                nc.scalar.activation(out=rstd, in_=mv[:, 1:2], func=AF.Sqrt,
                                     bias=eps_t, scale=1.0)
                nc.vector.reciprocal(out=rstd, in_=rstd)
                nmu = sm.tile([P, 1], F32, name="lnnmu", tag="lnnmu")
                nc.vector.tensor_scalar(out=nmu, in0=mv[:, 0:1], scalar1=rstd,
                                        scalar2=-1.0, op0=ALU.mult, op1=ALU.mult)
                nc.scalar.activation(out=ln[:, t, :], in_=src[:, t, :],
                                     func=AF.Identity, bias=nmu, scale=rstd)
            return ln

        def transpose_to(lnt, dst_pool, dst_tag, dt, idn):
            xt = dst_pool.tile([P, 8, TOK], dt, name=dst_tag, tag=dst_tag)
            with tc.tile_pool(name="tp_ps", bufs=2, space="PSUM") as tp_ps:
                for t in range(4):
                    for f in range(8):
                        pt = tp_ps.tile([P, P], lnt.dtype, name="tpt", tag="tpt",
                                        padded_shape=[P, 2 * P])
                        nc.tensor.transpose(pt[:, :], lnt[:, t, P * f:P * (f + 1)], idn)
                        eng = nc.vector if f % 2 == 0 else nc.scalar
                        if f % 2 == 0:
                            nc.vector.tensor_copy(out=xt[:, f, P * t:P * (t + 1)], in_=pt[:, :])
                        else:
                            nc.scalar.activation(out=xt[:, f, P * t:P * (t + 1)], in_=pt[:, :],
                                                 func=AF.Copy)
            return xt

        x2 = outer.tile([P, 4, C], F32)
        yT = outer.tile([P, 8, TOK], FP8D)

        with tc.tile_pool(name="mid", bufs=1) as mid:
            x_sb = mid.tile([P, 4, C], F32)
            for t in range(4):
                (nc.sync if t % 2 == 0 else nc.scalar).dma_start(
                    out=x_sb[:, t, :], in_=x_in[P * t:P * (t + 1), :])
            # qz: [part, sub(2), j, tok]  sub1 = zeros (DoubleRow zero-subtile)
            qz = mid.tile([P, 2, 8, TOK], FP8D)
            nc.gpsimd.memset(qz[:, 1, :, :], 0.0)

            # ================= qkv =================
            with tc.tile_pool(name="qkvp", bufs=1) as qp, \
                 tc.tile_pool(name="wqkv", bufs=2) as wp:
                ln1 = layer_norm(x_sb, qp, "ln")
                xT8 = transpose_to(ln1, qp, "xT8", FP8D, ident_bf)
                qkv_ps_cm = tc.tile_pool(name="qkv_ps", bufs=3, space="PSUM")
                qkv_ps = qkv_ps_cm.__enter__()

                wk_sb = wp.tile([P, 8, C], FP8D, name="wk", tag="wk")
                nc.scalar.dma_start(out=wk_sb, in_=wk_d[:, :].rearrange("(kc kp) n -> kp kc n", kp=P))
                # K^T feature tiles -> kt_in halves (K bias dropped: softmax-invariant)
                for f in range(8):
                    fo = P * f
                    ps = qkv_ps.tile([P, TOK], F32, name="kps", tag="qkvps")
                    for k in range(4):
                        for hh in range(2):
                            nc.tensor.matmul(ps[:, TOK // 2 * hh:TOK // 2 * (hh + 1)],
                                             wk_sb[:, 2 * k:2 * k + 2, fo:fo + P],
                                             xT8[:, 2 * k:2 * k + 2, 256 * hh:256 * (hh + 1)],
                                             start=(k == 0 and hh == 0),
                                             stop=(k == 3 and hh == 1), perf_mode=DR)
                    kt_sb = sm.tile([P, TOK], FP8D, name="kt_sb", tag="kt_sb", bufs=2)
                    nc.scalar.activation(out=kt_sb, in_=ps[:, :], func=AF.Copy)
                    for s in range(2):
                        nc.sync.dma_start(out=kt_in[s][P * f:P * (f + 1), :],
                                          in_=kt_sb[:, NCH * s:NCH * (s + 1)])
                # kt gathers fire as soon as K^T is written
                for s in range(2):
                    if mock_cc:
                        (nc.scalar if s == 0 else nc.sync).dma_start(
                            out=kt_all[s][0:C, :], in_=kt_in[s][:, :])
                    else:
                        nc.gpsimd.collective_compute("AllGather", ALU.bypass,
                                                     ins=[kt_in[s][:, :]], outs=[kt_all[s][:, :]],
                                                     replica_groups=RG)
                # V token tiles -> v_in halves (V bias folded into residual bias)
                wv_sb = wp.tile([P, 8, C], FP8D, name="wv", tag="wk")
                nc.sync.dma_start(out=wv_sb, in_=wv_d[:, :].rearrange("(kc kp) n -> kp kc n", kp=P))
                for t in range(4):
                    ps = qkv_ps.tile([P, C], F32, name="vps", tag="vps", bufs=2)
                    for k in range(4):
                        for n in range(4):
                            nc.tensor.matmul(ps[:, NCH * n:NCH * (n + 1)],
                                             xT8[:, 2 * k:2 * k + 2, P * t:P * (t + 1)],
                                             wv_sb[:, 2 * k:2 * k + 2, NCH * n:NCH * (n + 1)],
                                             start=(k == 0 and n % 2 == 0),
                                             stop=(k == 3 and n % 2 == 1), perf_mode=DR)
                    v_sb = sm.tile([P, C], FP8D, name="v_sb", tag="v_sb")
                    nc.scalar.activation(out=v_sb, in_=ps[:, :], func=AF.Copy)
                    sh, row = divmod(t, 2)
                    nc.sync.dma_start(out=v_in[sh][P * row:P * (row + 1), :], in_=v_sb)
                    if row == 1:
                        if mock_cc:
                            nc.gpsimd.dma_start(
                                out=v_all[sh][0:NCH, :], in_=v_in[sh][:, :])
                        else:
                            nc.gpsimd.collective_compute("AllGather", ALU.bypass,
                                                         ins=[v_in[sh][:, :]], outs=[v_all[sh][:, :]],
                                                         replica_groups=RG)

                # Q^T feature tiles (stay local); bias on copy, scale folded in exp
                wq_sb = wp.tile([P, 8, C], FP8D, name="wq", tag="wk")
                nc.sync.dma_start(out=wq_sb, in_=wq_d[:, :].rearrange("(kc kp) n -> kp kc n", kp=P))
                for f in range(8):
                    fo = P * f
                    ps = qkv_ps.tile([P, TOK], F32, name="qps", tag="qkvps")
                    for k in range(4):
                        for hh in range(2):
                            nc.tensor.matmul(ps[:, TOK // 2 * hh:TOK // 2 * (hh + 1)],
                                             wq_sb[:, 2 * k:2 * k + 2, fo:fo + P],
                                             xT8[:, 2 * k:2 * k + 2, 256 * hh:256 * (hh + 1)],
                                             start=(k == 0 and hh == 0),
                                             stop=(k == 3 and hh == 1), perf_mode=DR)
                    nc.scalar.activation(out=qz[:, 0, f, :], in_=ps[:, :], func=AF.Identity,
                                         bias=bq_sb[:, f:f + 1], scale=1.0)
                qkv_ps_cm.__exit__(None, None, None)

            # ============ attention (+ proj overlapped into phase B) ============
            with tc.tile_pool(name="attp", bufs=1) as ap, \
                 tc.tile_pool(name="projp", bufs=1) as pp, \
                 tc.tile_pool(name="pr_ps", bufs=2, space="PSUM") as pr_ps:
                wo_sb = pp.tile([P, 8, C], FP8D)
                nc.sync.dma_start(out=wo_sb, in_=wo_d[:, :].rearrange("(kc kp) n -> kp kc n", kp=P))
                for t in range(4):
                    nc.gpsimd.tensor_tensor(out=x_sb[:, t, :], in0=x_sb[:, t, :], in1=rb_bc, op=ALU.add)

                def load_kv(s):
                    # ktb: [part(2h d), kt-slot(8+1 pad), j, keys]
                    ktb = ap.tile([P, 9, 8, P], FP8D, name="ktb", tag="ktb", bufs=2)
                    nc.gpsimd.memset(ktb[:, 8, :, :], 0.0)
                    # vb: [part(key), kt-slot, hh, D+1]
                    vb = ap.tile([P, 8, 16, D + 1], FP8D, name="vb", tag="vb", bufs=2)
                    nc.vector.tensor_copy(out=vb[:, :, :, D:D + 1],
                                          in_=ones128.rearrange("p (a b) -> p a b", a=8)[:, :, 0:16])
                    for r in range(4):
                        nc.sync.dma_start(
                            out=ktb[:, 2 * r:2 * r + 2, :, :],
                            in_=kt_all[s][C * r:C * (r + 1), :].rearrange(
                                "(j p) (kb kc) -> p kb j kc", p=P, kb=2))
                        for sub in range(2):
                            nc.sync.dma_start(
                                out=vb[:, 2 * r + sub, :, 0:D],
                                in_=v_all[s][NCH * r + P * sub:NCH * r + P * (sub + 1), :]
                                        .rearrange("p (h d) -> p h d", h=H))
                    return ktb, vb

                def div_write(h, j, qc, ysrc, rsrc):
                    recip = sm.tile([1, NCH], F32, name=f"rc{h}", tag=f"rc{h}")
                    nc.vector.reciprocal(out=recip, in_=rsrc)
                    rb = sm.tile([D, NCH], F32, name=f"rb{h}", tag=f"rb{h}")
                    nc.gpsimd.partition_broadcast(rb, recip)
                    nc.vector.tensor_tensor(out=yT[64 * h:64 * (h + 1), j, NCH * qc:NCH * (qc + 1)],
                                            in0=ysrc, in1=rb, op=ALU.mult)

                def proj(trange):
                    for t in trange:
                        for nn in range(2):
                            ps = pr_ps.tile([P, 512], F32, name="prps", tag="prps")
                            for k in range(4):
                                for n2 in range(2):
                                    nc.tensor.matmul(
                                        ps[:, NCH * n2:NCH * (n2 + 1)],
                                        yT[:, 2 * k:2 * k + 2, P * t:P * (t + 1)],
                                        wo_sb[:, 2 * k:2 * k + 2, 512 * nn + NCH * n2:512 * nn + NCH * (n2 + 1)],
                                        start=(k == 0 and n2 == 0),
                                        stop=(k == 3 and n2 == 1), perf_mode=DR)
                            nc.vector.tensor_tensor(out=x2[:, t, 512 * nn:512 * (nn + 1)], in0=ps[:, :],
                                                    in1=x_sb[:, t, 512 * nn:512 * (nn + 1)], op=ALU.add)

                def score_block(j, s, kt, st, ktb, qcs):
                    """st: psum [P, 2h, len(qcs), NCH]. Emits scores + tri for kt."""
                    par = kt % 2
                    for h in range(2):
                        for ci, qc in enumerate(qcs):
                            sl = slot(s, kt, qc)
                            nc.tensor.matmul(
                                st[:, h, ci, :],
                                ktb[64 * h:64 * (h + 1), kt:kt + 2, j, :],
                                qz[64 * h:64 * (h + 1), :, j, NCH * qc:NCH * (qc + 1)],
                                start=(ci == 0), stop=(ci == len(qcs) - 1),
                                perf_mode=DR, tile_position=(64 * h, 0))
                            # diagonal triangle: -256 into the masked region
                            nc.tensor.matmul(
                                st[:, h, ci, P * par:P * (par + 1)],
                                negI[:, :], mrhs[:, sl, :],
                                start=False, stop=False, skip_group_check=True)
                            if par == 1:
                                # odd diagonal block: first q-half fully dead
                                nc.tensor.matmul(
                                    st[:, h, ci, 0:P],
                                    negI[:, :], mkill[:, sl, :],
                                    start=False, stop=False, skip_group_check=True)

                def exp_block(j, s, kt, st, et, kslot, qcs):
                    for ci, qc in enumerate(qcs):
                        sl = slot(s, kt, qc)
                        nc.scalar.activation(
                            out=et[:, kslot, :, ci, :], in_=st[:, :, ci, :],
                            func=AF.Exp, scale=SCALE, bias=be[:, sl:sl + 1])

                def av_block(j, pair, et, vb, ya, cis, starts, stops):
                    for h in range(2):
                        for ei, (ci, start, stop) in enumerate(zip(cis, starts, stops)):
                            nc.tensor.matmul(
                                ya[h][:, ci, :],
                                vb[:, 2 * pair:2 * pair + 2, 2 * j + h, :],
                                et[:, :, h, ei, :],
                                start=start, stop=stop, perf_mode=DR)

                # ---- load both phases up front ----
                ktbA, vbA = load_kv(0)
                ktbB, vbB = load_kv(1)
                with tc.tile_pool(name="at_ps0", bufs=1, space="PSUM") as at_ps:
                    for j in range(8):
                        ya = [at_ps.tile([D + 1, 2, NCH], F32, name=f"ya{h}", tag=f"ya{h}", bufs=1)
                              for h in range(2)]
                        # ---- phase A: keys 0:1024, both chunks ----
                        for pair in range(4):
                            et = sm.tile([P, 2, 2, 2, NCH], FP8D, name="et", tag="et", bufs=2)
                            for kslot in range(2):
                                kt = 2 * pair + kslot
                                st = at_ps.tile([P, 2, 2, NCH], F32, name="st", tag="st",
                                                bufs=2)
                                score_block(j, 0, kt, st, ktbA, (0, 1))
                                exp_block(j, 0, kt, st, et, kslot, (0, 1))
                            av_block(j, pair, et, vbA, ya, (0, 1),
                                     starts=(pair == 0, False),
                                     stops=(False, False))
                        for h in range(2):
                            div_write(h, j, 0, ya[h][0:D, 0, :], ya[h][D:D + 1, 0, :])
                        # ---- phase B: keys 1024:2048, chunk 1 only ----
                        for pair in range(4):
                            etb = sm.tile([P, 2, 2, 1, NCH], FP8D, name="etb", tag="etb", bufs=2)
                            for kslot in range(2):
                                kt = 2 * pair + kslot
                                stb = at_ps.tile([P, 2, 2, NCH], F32, name="st", tag="st",
                                                 bufs=2)
                                score_block(j, 1, kt, stb, ktbB, (1,))
                                exp_block(j, 1, kt, stb, etb, kslot, (1,))
                            av_block(j, pair, etb, vbB, ya, (1,),
                                     starts=(False,), stops=(pair == 3,))
                        for h in range(2):
                            div_write(h, j, 1, ya[h][0:D, 1, :], ya[h][D:D + 1, 1, :])

                proj([0, 1, 2, 3])

        # ================= LN2 + MLP =================
        with tc.tile_pool(name="mlpp", bufs=1) as mp, \
             tc.tile_pool(name="wmlp", bufs=3) as wmp:
            ln2 = layer_norm(x2, mp, "ln2", dt=BF16)
            xln2T = transpose_to(ln2, mp, "xln2T", dt=FC1D, idn=ident_bf)
            for t in range(4):
                nc.gpsimd.tensor_tensor(out=x2[:, t, :], in0=x2[:, t, :], in1=b2_bc, op=ALU.add)

            h_sb = mp.tile([P, 32, 512], FC2D)
            for half in range(2):
                with tc.tile_pool(name=f"mlp_ps{half}", bufs=1, space="PSUM") as mlp_ps:
                    ops = [mlp_ps.tile([P, 512], F32, name=f"ops{t}", tag=f"ops{t}", bufs=1)
                           for t in range(4)]
                    for m in range(32):
                        if half == 0:
                            if m % 4 == 0:
                                wfc = wmp.tile([P, 8, 512], FC1D, name="wfc", tag="wfc")
                                nc.sync.dma_start(out=wfc,
                                                    in_=wfc_d[:, 512 * (m // 4):512 * (m // 4 + 1)]
                                                    .rearrange("(kc kp) n -> kp kc n", kp=P))
                            mo = P * (m % 4)
                            fps = mlp_ps.tile([P, 512], F32, name="fps", tag="fps", bufs=4)
                            if FP8_FC1:
                                for k in range(4):
                                    for hh in range(2):
                                        nc.tensor.matmul(fps[:, 256 * hh:256 * (hh + 1)],
                                                         wfc[:, 2 * k:2 * k + 2, mo:mo + P],
                                                         xln2T[:, 2 * k:2 * k + 2, 256 * hh:256 * (hh + 1)],
                                                         start=(k == 0 and hh == 0),
                                                         stop=(k == 3 and hh == 1), perf_mode=DR)
                            else:
                                for k in range(8):
                                    nc.tensor.matmul(fps[:, :], wfc[:, k, mo:mo + P], xln2T[:, k, :],
                                                     start=(k == 0), stop=(k == 7))
                            nc.scalar.activation(out=h_sb[:, m, :], in_=fps[:, :], func=AF.Gelu,
                                                 bias=bfc_sb[:, m:m + 1], scale=1.0)
                        if m % 4 == 0:
                            w2 = wmp.tile([P, 4, 512], FC2D, name="w2", tag="w2", bufs=3)
                            nc.sync.dma_start(out=w2, in_=wfc2_d[P * m:P * (m + 4),
                                                               512 * half:512 * (half + 1)]
                                                .rearrange("(mc mp) n -> mp mc n", mp=P))
                        if FP8_FC2:
                            if m % 2 == 0:
                                for t in range(4):
                                    for n in range(2):
                                        nc.tensor.matmul(
                                            ops[t][:, 256 * n:256 * (n + 1)],
                                            h_sb[:, m:m + 2, P * t:P * (t + 1)],
                                            w2[:, (m % 4):(m % 4) + 2, 256 * n:256 * (n + 1)],
                                            start=(m == 0 and n == 0),
                                            stop=(m == 30 and n == 1), perf_mode=DR)
                        else:
                            for t in range(4):
                                nc.tensor.matmul(ops[t][:, :], h_sb[:, m, P * t:P * (t + 1)],
                                                 w2[:, m % 4, :], start=(m == 0), stop=(m == 31))
                    for t in range(4):
                        nc.vector.tensor_tensor(out=x2[:, t, 512 * half:512 * (half + 1)],
                                                in0=ops[t][:, :],
                                                in1=x2[:, t, 512 * half:512 * (half + 1)], op=ALU.add)
                        if half == 1:
                            nc.sync.dma_start(out=out_ext[P * t:P * (t + 1), :], in_=x2[:, t, :])

    nc.finalize()
    return nc


def _get_nc():
    if "nc" not in _CACHE:
        _CACHE["nc"] = _build()
    return _CACHE["nc"]


def _prep(**inputs):
    f = lambda a: np.asarray(a, dtype=np.float32)
    x = f(inputs["x"])
    ln1_g, ln1_b = f(inputs["ln1_g"]), f(inputs["ln1_b"])
    ln2_g, ln2_b = f(inputs["ln2_g"]), f(inputs["ln2_b"])
    W_attn, b_attn = f(inputs["W_attn"]), f(inputs["b_attn"])
    W_o, b_o = f(inputs["W_o"]), f(inputs["b_o"])
    W_fc, b_fc = f(inputs["W_fc"]), f(inputs["b_fc"])
    W_fc2, b_fc2 = f(inputs["W_fc2"]), f(inputs["b_fc2"])

    # fold LN affine params into the next matmul
    W_attn_e = ln1_g[:, None] * W_attn
    b_attn_e = b_attn + ln1_b @ W_attn
    W_fc_e = ln2_g[:, None] * W_fc
    b_fc_e = b_fc + ln2_b @ W_fc
    # V bias contributes a constant through attention: fold b_v @ W_o into
    # the residual bias (K bias is softmax-invariant and dropped).
    rb = b_o + b_attn_e[2 * C:3 * C] @ W_o

    fc1d = FP8 if FP8_FC1 else ml_dtypes.bfloat16
    fc2d = FP8 if FP8_FC2 else ml_dtypes.bfloat16

    in_maps = []
    for r in range(N_CORES):
        b, p = divmod(r, 4)
        c0, c1 = p, 7 - p
        xs = np.concatenate([x[b, NCH * c0:NCH * (c0 + 1)],
                             x[b, NCH * c1:NCH * (c1 + 1)]], axis=0)
        in_maps.append({
            "x": np.ascontiguousarray(xs),
            "qbase": np.array([[NCH * c0, NCH * c1]], dtype=np.float32),
            "wq": W_attn_e[:, 0:C].astype(FP8),
            "wk": W_attn_e[:, C:2 * C].astype(FP8),
            "wv": W_attn_e[:, 2 * C:3 * C].astype(FP8),
            "bq": b_attn_e[0:C],
            "wo": W_o.astype(FP8), "rb": rb,
            "w_fc": W_fc_e.astype(fc1d), "b_fc": b_fc_e,
            "w_fc2": W_fc2.astype(fc2d), "b_fc2": b_fc2,
        })

    def assemble(results):
        out = np.empty((B, T, C), dtype=np.float32)
        for r in range(N_CORES):
            b, p = divmod(r, 4)
            c0, c1 = p, 7 - p
            o = results[r]["out"]
            out[b, NCH * c0:NCH * (c0 + 1)] = o[0:NCH]
            out[b, NCH * c1:NCH * (c1 + 1)] = o[NCH:TOK]
        return out

    return in_maps, assemble


def kernel(**inputs):
    from concourse.bass_utils import run_bass_kernel_spmd

    in_maps, assemble = _prep(**inputs)
    res = run_bass_kernel_spmd(_get_nc(), in_maps, list(range(N_CORES)))
    return assemble(res.results)


# revision 20
# speedup vs baseline: 1.6079x; 1.0591x over previous
"""Transformer block (pre-LN causal MHA + GELU MLP) on 8 trn2 NeuronCores.

Sharding: core r handles batch b=r//4, group position p=r%4, owning token
chunks {p, 7-p} of eight 256-token chunks (causally balanced zigzag).
Sequence-parallel everywhere except attention: K^T and V for the full batch
are exchanged via fp8 AllGathers inside each 4-core batch group.

All heavy matmuls run in fp8e4 with DoubleRow perf mode (2 contraction
k-tiles per instruction at 0.5 cycles/row): QKV projections, attention
scores (K=64 with a zeroed second subtile on the Q side), attention*V
(key-tile pairs), output projection, and the MLP (precision tier
selectable per matmul via the FP8_* flags).

Masking is done on the PE + Act engines instead of element-wise DVE
multiplies: fully-masked (key-block, chunk) tiles get exp bias -30 from a
data-driven per-tile bias table (exp underflows to 0 in fp8), and the two
diagonal key-blocks per chunk get -256 added to the masked triangle via a
single extra matmul (lhsT=-256*I, rhs=triangle indicator built from qbase)
before the exp. Scores carry no 1/sqrt(d) or softmax-max handling: the
scale (0.125) and a -4*ln2 range shift are folded into the exp activation
(exp output ~ exp(s)/16 stays within fp8e4 range; the shift cancels in the
softmax division).

Bias handling: K bias is dropped (softmax is invariant to per-query score
shifts), V bias is folded into the residual bias on the host
(b_o + b_v @ W_o), Q bias is applied on the PSUM->SBUF copy, fc bias rides
the GELU activation, fc2 bias is pre-added to the residual.

LN rsqrt = exp(-0.5*ln(var+eps)) so LN1/attention/LN2 share one activation
table (natural_log_exp) and only the MLP's gelu forces a table switch.

Self-contained: hardcodes B=2, T=2048, C=1024, H=16, D=64, hidden=4096.
"""
import sys

if "/opt/trn_rl_repo" not in sys.path:
    sys.path.insert(0, "/opt/trn_rl_repo")

import numpy as np
import ml_dtypes

B, T, C, H = 2, 2048, 1024, 16
D = C // H            # 64
MH = 4 * C            # 4096 mlp hidden
EPS = 1e-5
P = 128
TOK = 512             # tokens per core
NCH = 256             # tokens per chunk
N_CORES = 8
EXPB = -2.7725887     # -4*ln2: exp emits exp(s)/16
SCALE = 0.125         # 1/sqrt(D)

# precision tiers (fp8 DoubleRow vs bf16) — tuned empirically
FP8_FC1 = True
FP8_FC2 = True

FP8 = ml_dtypes.float8_e4m3

_CACHE: dict = {}


def _build(mock_cc=False):
    import concourse.tile as tile
    from concourse import bacc, mybir
    from concourse.masks import make_identity
    from contextlib import ExitStack

    F32 = mybir.dt.float32
    BF16 = mybir.dt.bfloat16
    FP8D = mybir.dt.float8e4
    I32 = mybir.dt.int32
    AF = mybir.ActivationFunctionType
    ALU = mybir.AluOpType
    DR = mybir.MatmulPerfMode.DoubleRow

    FC1D = FP8D if FP8_FC1 else BF16
    FC2D = FP8D if FP8_FC2 else BF16

    nc = bacc.Bacc()

    # ---------------- I/O ----------------
    x_in = nc.declare_dram_parameter("x", [TOK, C], F32, isOutput=False)
    qbase_in = nc.declare_dram_parameter("qbase", [1, 2], F32, isOutput=False)
    wq_d = nc.declare_dram_parameter("wq", [C, C], FP8D, isOutput=False)
    wk_d = nc.declare_dram_parameter("wk", [C, C], FP8D, isOutput=False)
    wv_d = nc.declare_dram_parameter("wv", [C, C], FP8D, isOutput=False)
    bq_d = nc.declare_dram_parameter("bq", [C], F32, isOutput=False)
    wo_d = nc.declare_dram_parameter("wo", [C, C], FP8D, isOutput=False)
    rb_d = nc.declare_dram_parameter("rb", [C], F32, isOutput=False)
    wfc_d = nc.declare_dram_parameter("w_fc", [C, MH], FC1D, isOutput=False)
    bfc_d = nc.declare_dram_parameter("b_fc", [MH], F32, isOutput=False)
    wfc2_d = nc.declare_dram_parameter("w_fc2", [MH, C], FC2D, isOutput=False)
    bfc2_d = nc.declare_dram_parameter("b_fc2", [C], F32, isOutput=False)
    out_ext = nc.declare_dram_parameter("out", [TOK, C], F32, isOutput=True)

    # internal DRAM for the collectives (half s=0: keys 0:1024, s=1: 1024:2048)
    kt_in = [nc.dram_tensor(f"kt_in_{s}", [C, NCH], FP8D) for s in range(2)]
    v_in = [nc.dram_tensor(f"v_in_{s}", [NCH, C], FP8D) for s in range(2)]
    kt_all = [nc.dram_tensor(f"kt_all_{s}", [4 * C, NCH], FP8D) for s in range(2)]
    v_all = [nc.dram_tensor(f"v_all_{s}", [4 * NCH, C], FP8D) for s in range(2)]
    stash_d = nc.dram_tensor("stash_d", [16, D + 1, NCH], F32)
    RG = [[0, 1, 2, 3], [4, 5, 6, 7]]

    with tile.TileContext(nc) as tc, ExitStack() as ctx:
        # ---------- pools: outer (whole kernel) ----------
        const = ctx.enter_context(tc.tile_pool(name="const", bufs=1))
        outer = ctx.enter_context(tc.tile_pool(name="outer", bufs=1))
        sm = ctx.enter_context(tc.tile_pool(name="sm", bufs=2))

        # ---------- constants ----------
        ident = const.tile([P, P], F32)
        make_identity(nc, ident)
        ident_bf = const.tile([P, P], BF16)
        nc.vector.tensor_copy(out=ident_bf, in_=ident)
        eps_t = const.tile([P, 1], F32)
        nc.vector.memset(eps_t, EPS)
        expb_t = const.tile([P, 1], F32)
        nc.vector.memset(expb_t, EXPB)
        ones128 = const.tile([P, P], F32)
        nc.vector.memset(ones128, 1.0)
        # -128 * I in fp8 (tri-mask stationary operand; exp(s/8-16) -> 0 in fp8)
        negI = const.tile([P, P], FP8D)
        negI_f = const.tile([P, P], F32)
        nc.vector.tensor_scalar(out=negI_f, in0=ident, scalar1=-128.0, scalar2=None,
                                op0=ALU.mult)
        nc.vector.tensor_copy(out=negI, in_=negI_f)

        bq_sb = const.tile([P, 8], F32)     # q bias -> [128, 8]
        nc.sync.dma_start(out=bq_sb, in_=bq_d[0:C].rearrange("(f p) -> p f", p=P))
        bfc_sb = const.tile([P, 32], F32)
        nc.sync.dma_start(out=bfc_sb, in_=bfc_d[:].rearrange("(f p) -> p f", p=P))
        rb_bc = const.tile([P, C], F32)     # residual bias (b_o + b_v@W_o) bcast
        nc.sync.dma_start(out=rb_bc, in_=rb_d[:].rearrange("(a c) -> a c", a=1).to_broadcast((P, C)))
        b2_bc = const.tile([P, C], F32)
        nc.sync.dma_start(out=b2_bc, in_=bfc2_d[:].rearrange("(a c) -> a c", a=1).to_broadcast((P, C)))

        # qbase + iotas for mask tables
        qbase_sb = const.tile([1, 2], F32)
        nc.sync.dma_start(out=qbase_sb, in_=qbase_in[:, :])
        kidx_i = const.tile([P, 1], I32)
        nc.gpsimd.iota(kidx_i, pattern=[[0, 1]], base=0, channel_multiplier=1)
        kidx_f = const.tile([P, 1], F32)
        nc.vector.tensor_copy(out=kidx_f, in_=kidx_i)
        qio_i = const.tile([1, P], I32)
        nc.gpsimd.iota(qio_i, pattern=[[1, P]], base=0, channel_multiplier=0)
        qio_f = const.tile([1, P], F32)
        nc.vector.tensor_copy(out=qio_f, in_=qio_i)
        # TRI[k, q] = 1 if q < k else 0  (masked region of an aligned 128-diag)
        qio_bc = const.tile([P, P], F32)
        nc.gpsimd.partition_broadcast(qio_bc, qio_f)
        tri_f = const.tile([P, P], F32)
        nc.vector.tensor_scalar(out=tri_f, in0=qio_bc, scalar1=kidx_f, scalar2=None,
                                op0=ALU.is_lt)

        # ---- per-(phase, kt, chunk) exp bias table: alive -> EXPB, dead -> -30
        # slot order: (s, kt, c) -> 32 slots (s in 0..1, kt 0..7, c 0..1)
        kb_i = const.tile([1, 32], I32)
        nc.gpsimd.iota(kb_i, pattern=[[1024, 2], [128, 8], [0, 2]], base=0,
                       channel_multiplier=0)
        kb_f = const.tile([1, 32], F32)
        nc.vector.tensor_copy(out=kb_f, in_=kb_i)
        csel_i = const.tile([1, 32], I32)   # 0,1,0,1,... chunk selector
        nc.gpsimd.iota(csel_i, pattern=[[0, 2], [0, 8], [1, 2]], base=0,
                       channel_multiplier=0)
        csel_f = const.tile([1, 32], F32)
        nc.vector.tensor_copy(out=csel_f, in_=csel_i)
        # qb_slot = qbase[c0] + csel*(qbase[c1]-qbase[c0])
        qdiff = const.tile([1, 1], F32)
        nc.vector.tensor_scalar(out=qdiff, in0=qbase_sb[0:1, 1:2],
                                scalar1=qbase_sb[0:1, 0:1], scalar2=None,
                                op0=ALU.subtract)
        qb_slot = const.tile([1, 32], F32)
        nc.vector.tensor_scalar(out=qb_slot, in0=csel_f, scalar1=qdiff,
                                scalar2=qbase_sb[0:1, 0:1], op0=ALU.mult, op1=ALU.add)
        # alive = (qb_slot + 255 >= kb)  <=>  qb_slot - kb >= -255
        alive = const.tile([1, 32], F32)
        nc.vector.tensor_tensor(out=alive, in0=qb_slot, in1=kb_f, op=ALU.subtract)
        nc.vector.tensor_scalar(out=alive, in0=alive, scalar1=-255.0, scalar2=None,
                                op0=ALU.is_ge)
        be_row = const.tile([1, 32], F32)   # -30 + alive*(30+EXPB)
        nc.vector.tensor_scalar(out=be_row, in0=alive, scalar1=30.0 + EXPB,
                                scalar2=-30.0, op0=ALU.mult, op1=ALU.add)
        be = const.tile([P, 32], F32)
        nc.gpsimd.partition_broadcast(be, be_row)

        # ---- tri-mask rhs table: mrhs[:, slot, :] = TRI * diag(slot)
        # diag(slot) = 1 iff kb[slot] == qb_slot + 128*parity(kt)
        par_i = const.tile([1, 32], I32)
        nc.gpsimd.iota(par_i, pattern=[[0, 2], [0, 4], [128, 2], [0, 2]], base=0,
                       channel_multiplier=0)   # (s, ktpair, par, c) -> 128*(kt%2)
        par_f = const.tile([1, 32], F32)
        nc.vector.tensor_copy(out=par_f, in_=par_i)
        dfl = const.tile([1, 32], F32)
        nc.vector.tensor_tensor(out=dfl, in0=kb_f, in1=par_f, op=ALU.subtract)
        nc.vector.tensor_tensor(out=dfl, in0=dfl, in1=qb_slot, op=ALU.is_equal)
        dflb = const.tile([P, 32], F32)
        nc.gpsimd.partition_broadcast(dflb, dfl)
        mrhs = const.tile([P, 32, P], FP8D)
        for sl in range(32):
            nc.vector.tensor_scalar(out=mrhs[:, sl, :], in0=tri_f,
                                    scalar1=dflb[:, sl:sl + 1], scalar2=None,
                                    op0=ALU.mult)
        # full-kill pattern for the odd diagonal block's dead first q-half
        mkill = const.tile([P, 32, P], FP8D)
        for sl in range(32):
            if (sl // 2) % 2 == 1:   # odd kt slots only
                nc.vector.tensor_scalar(out=mkill[:, sl, :], in0=ones128,
                                        scalar1=dflb[:, sl:sl + 1], scalar2=None,
                                        op0=ALU.mult)
        # dead = 1 - alive, broadcast per slot
        deadb = const.tile([P, 32], F32)
        nc.vector.tensor_scalar(out=deadb, in0=dflb, scalar1=0.0, scalar2=None,
                                op0=ALU.mult)   # placeholder shape; overwritten
        dead_row = const.tile([1, 32], F32)
        nc.vector.tensor_scalar(out=dead_row, in0=alive, scalar1=-1.0,
                                scalar2=1.0, op0=ALU.mult, op1=ALU.add)
        nc.gpsimd.partition_broadcast(deadb, dead_row)
        ones2 = const.tile([P, 2, NCH], F32)
        nc.vector.memset(ones2, 1.0)
        dkill = const.tile([P, 6, 2, NCH], FP8D)
        for i in range(6):
            sl = 2 * (i + 2)   # slot(0, kt=i+2, c=0)
            nc.vector.tensor_scalar(out=dkill[:, i, :, :], in0=ones2,
                                    scalar1=deadb[:, sl:sl + 1], scalar2=None,
                                    op0=ALU.mult)

        def slot(s, kt, c):
            return s * 16 + kt * 2 + c

        # ---------- helpers ----------
        def layer_norm(src, dst_pool, tag, dt=BF16):
            # per-t sqrt (Sqrt table shared across all 4 calls; Exp/Gelu load later)
            ln = dst_pool.tile([P, 4, C], dt, name=tag, tag=tag)
            for t in range(4):
                stats = sm.tile([P, 2, 6], F32, name="lnstats", tag="lnstats")
                nc.vector.bn_stats(out=stats[:, 0, :], in_=src[:, t, 0:512])
                nc.vector.bn_stats(out=stats[:, 1, :], in_=src[:, t, 512:1024])
                mv = sm.tile([P, 2], F32, name="lnmv", tag="lnmv")
                nc.vector.bn_aggr(out=mv, in_=stats)
                rstd = sm.tile([P, 1], F32, name="lnrstd", tag="lnrstd")
                nc.scalar.activation(out=rstd, in_=mv[:, 1:2], func=AF.Sqrt,
                                     bias=eps_t, scale=1.0)
                nc.vector.reciprocal(out=rstd, in_=rstd)
                nmu = sm.tile([P, 1], F32, name="lnnmu", tag="lnnmu")
                nc.vector.tensor_scalar(out=nmu, in0=mv[:, 0:1], scalar1=rstd,
                                        scalar2=-1.0, op0=ALU.mult, op1=ALU.mult)
                nc.scalar.activation(out=ln[:, t, :], in_=src[:, t, :],
                                     func=AF.Identity, bias=nmu, scale=rstd)
            return ln

        def transpose_to(lnt, dst_pool, dst_tag, dt, idn):
            xt = dst_pool.tile([P, 8, TOK], dt, name=dst_tag, tag=dst_tag)
            with tc.tile_pool(name="tp_ps", bufs=2, space="PSUM") as tp_ps:
                for t in range(4):
                    for f in range(8):
                        pt = tp_ps.tile([P, P], lnt.dtype, name="tpt", tag="tpt",
                                        padded_shape=[P, 2 * P])
                        nc.tensor.transpose(pt[:, :], lnt[:, t, P * f:P * (f + 1)], idn)
                        eng = nc.vector if f % 2 == 0 else nc.scalar
                        if f % 2 == 0:
                            nc.vector.tensor_copy(out=xt[:, f, P * t:P * (t + 1)], in_=pt[:, :])
                        else:
                            nc.scalar.activation(out=xt[:, f, P * t:P * (t + 1)], in_=pt[:, :],
                                                 func=AF.Copy)
            return xt

        x2 = outer.tile([P, 4, C], F32)
        yT = outer.tile([P, 8, TOK], FP8D)

        with tc.tile_pool(name="mid", bufs=1) as mid:
            x_sb = mid.tile([P, 4, C], F32)
            for t in range(4):
                (nc.sync if t % 2 == 0 else nc.scalar).dma_start(
                    out=x_sb[:, t, :], in_=x_in[P * t:P * (t + 1), :])
            # qz: [part, sub(2), j, tok]  sub1 = zeros (DoubleRow zero-subtile)
            qz = mid.tile([P, 2, 8, TOK], FP8D)
            nc.gpsimd.memset(qz[:, 1, :, :], 0.0)

            # ================= qkv =================
            with tc.tile_pool(name="qkvp", bufs=1) as qp, \
                 tc.tile_pool(name="wqkv", bufs=2) as wp:
                ln1 = layer_norm(x_sb, qp, "ln")
                xT8 = transpose_to(ln1, qp, "xT8", FP8D, ident_bf)
                qkv_ps_cm = tc.tile_pool(name="qkv_ps", bufs=3, space="PSUM")
                qkv_ps = qkv_ps_cm.__enter__()

                wk_sb = wp.tile([P, 8, C], FP8D, name="wk", tag="wk")
                nc.scalar.dma_start(out=wk_sb, in_=wk_d[:, :].rearrange("(kc kp) n -> kp kc n", kp=P))
                # K^T feature tiles -> kt_in halves (K bias dropped: softmax-invariant)
                for f in range(8):
                    fo = P * f
                    ps = qkv_ps.tile([P, TOK], F32, name="kps", tag="qkvps")
                    for k in range(4):
                        for hh in range(2):
                            nc.tensor.matmul(ps[:, TOK // 2 * hh:TOK // 2 * (hh + 1)],
                                             wk_sb[:, 2 * k:2 * k + 2, fo:fo + P],
                                             xT8[:, 2 * k:2 * k + 2, 256 * hh:256 * (hh + 1)],
                                             start=(k == 0 and hh == 0),
                                             stop=(k == 3 and hh == 1), perf_mode=DR)
                    kt_sb = sm.tile([P, TOK], FP8D, name="kt_sb", tag="kt_sb", bufs=2)
                    nc.scalar.activation(out=kt_sb, in_=ps[:, :], func=AF.Copy)
                    for s in range(2):
                        nc.sync.dma_start(out=kt_in[s][P * f:P * (f + 1), :],
                                          in_=kt_sb[:, NCH * s:NCH * (s + 1)])
                # kt gathers fire as soon as K^T is written
                for s in range(2):
                    if mock_cc:
                        (nc.scalar if s == 0 else nc.sync).dma_start(
                            out=kt_all[s][0:C, :], in_=kt_in[s][:, :])
                    else:
                        nc.gpsimd.collective_compute("AllGather", ALU.bypass,
                                                     ins=[kt_in[s][:, :]], outs=[kt_all[s][:, :]],
                                                     replica_groups=RG)
                # V token tiles -> v_in halves (V bias folded into residual bias)
                wv_sb = wp.tile([P, 8, C], FP8D, name="wv", tag="wk")
                nc.sync.dma_start(out=wv_sb, in_=wv_d[:, :].rearrange("(kc kp) n -> kp kc n", kp=P))
                for t in range(4):
                    ps = qkv_ps.tile([P, C], F32, name="vps", tag="vps", bufs=2)
                    for k in range(4):
                        for n in range(4):
                            nc.tensor.matmul(ps[:, NCH * n:NCH * (n + 1)],
                                             xT8[:, 2 * k:2 * k + 2, P * t:P * (t + 1)],
                                             wv_sb[:, 2 * k:2 * k + 2, NCH * n:NCH * (n + 1)],
                                             start=(k == 0 and n % 2 == 0),
                                             stop=(k == 3 and n % 2 == 1), perf_mode=DR)
                    v_sb = sm.tile([P, C], FP8D, name="v_sb", tag="v_sb")
                    nc.scalar.activation(out=v_sb, in_=ps[:, :], func=AF.Copy)
                    sh, row = divmod(t, 2)
                    nc.sync.dma_start(out=v_in[sh][P * row:P * (row + 1), :], in_=v_sb)
                    if row == 1:
                        if mock_cc:
                            nc.gpsimd.dma_start(
                                out=v_all[sh][0:NCH, :], in_=v_in[sh][:, :])
                        else:
                            nc.gpsimd.collective_compute("AllGather", ALU.bypass,
                                                         ins=[v_in[sh][:, :]], outs=[v_all[sh][:, :]],
                                                         replica_groups=RG)

                # Q^T feature tiles (stay local); bias on copy, scale folded in exp
                wq_sb = wp.tile([P, 8, C], FP8D, name="wq", tag="wk")
                nc.sync.dma_start(out=wq_sb, in_=wq_d[:, :].rearrange("(kc kp) n -> kp kc n", kp=P))
                for f in range(8):
                    fo = P * f
                    ps = qkv_ps.tile([P, TOK], F32, name="qps", tag="qkvps")
                    for k in range(4):
                        for hh in range(2):
                            nc.tensor.matmul(ps[:, TOK // 2 * hh:TOK // 2 * (hh + 1)],
                                             wq_sb[:, 2 * k:2 * k + 2, fo:fo + P],
                                             xT8[:, 2 * k:2 * k + 2, 256 * hh:256 * (hh + 1)],
                                             start=(k == 0 and hh == 0),
                                             stop=(k == 3 and hh == 1), perf_mode=DR)
                    nc.scalar.activation(out=qz[:, 0, f, :], in_=ps[:, :], func=AF.Identity,
                                         bias=bq_sb[:, f:f + 1], scale=1.0)
                qkv_ps_cm.__exit__(None, None, None)

            # ============ attention (+ proj overlapped into phase B) ============
            with tc.tile_pool(name="attp", bufs=1) as ap, \
                 tc.tile_pool(name="projp", bufs=1) as pp, \
                 tc.tile_pool(name="pr_ps", bufs=2, space="PSUM") as pr_ps:
                wo_sb = pp.tile([P, 8, C], FP8D)
                nc.sync.dma_start(out=wo_sb, in_=wo_d[:, :].rearrange("(kc kp) n -> kp kc n", kp=P))
                for t in range(4):
                    nc.gpsimd.tensor_tensor(out=x_sb[:, t, :], in0=x_sb[:, t, :], in1=rb_bc, op=ALU.add)

                def load_kv(s):
                    # ktb: [part(2h d), kt-slot(8+1 pad), j, keys]
                    ktb = ap.tile([P, 9, 8, P], FP8D, name="ktb", tag="ktb", bufs=2)
                    nc.gpsimd.memset(ktb[:, 8, :, :], 0.0)
                    # vb: [part(key), kt-slot, hh, D+1]
                    vb = ap.tile([P, 8, 16, D + 1], FP8D, name="vb", tag="vb", bufs=2)
                    nc.vector.tensor_copy(out=vb[:, :, :, D:D + 1],
                                          in_=ones128.rearrange("p (a b) -> p a b", a=8)[:, :, 0:16])
                    for r in range(4):
                        nc.sync.dma_start(
                            out=ktb[:, 2 * r:2 * r + 2, :, :],
                            in_=kt_all[s][C * r:C * (r + 1), :].rearrange(
                                "(j p) (kb kc) -> p kb j kc", p=P, kb=2))
                        for sub in range(2):
                            nc.sync.dma_start(
                                out=vb[:, 2 * r + sub, :, 0:D],
                                in_=v_all[s][NCH * r + P * sub:NCH * r + P * (sub + 1), :]
                                        .rearrange("p (h d) -> p h d", h=H))
                    return ktb, vb

                def div_write(h, j, qc, ysrc, rsrc):
                    recip = sm.tile([1, NCH], F32, name=f"rc{h}", tag=f"rc{h}")
                    nc.vector.reciprocal(out=recip, in_=rsrc)
                    rb = sm.tile([D, NCH], F32, name=f"rb{h}", tag=f"rb{h}")
                    nc.gpsimd.partition_broadcast(rb, recip)
                    nc.vector.tensor_tensor(out=yT[64 * h:64 * (h + 1), j, NCH * qc:NCH * (qc + 1)],
                                            in0=ysrc, in1=rb, op=ALU.mult)

                def proj(trange):
                    for t in trange:
                        for nn in range(2):
                            ps = pr_ps.tile([P, 512], F32, name="prps", tag="prps")
                            for k in range(4):
                                for n2 in range(2):
                                    nc.tensor.matmul(
                                        ps[:, NCH * n2:NCH * (n2 + 1)],
                                        yT[:, 2 * k:2 * k + 2, P * t:P * (t + 1)],
                                        wo_sb[:, 2 * k:2 * k + 2, 512 * nn + NCH * n2:512 * nn + NCH * (n2 + 1)],
                                        start=(k == 0 and n2 == 0),
                                        stop=(k == 3 and n2 == 1), perf_mode=DR)
                            nc.vector.tensor_tensor(out=x2[:, t, 512 * nn:512 * (nn + 1)], in0=ps[:, :],
                                                    in1=x_sb[:, t, 512 * nn:512 * (nn + 1)], op=ALU.add)

                def score_block(j, s, kt, st, ktb, qcs):
                    """st: psum [P, 2h, len(qcs), NCH]. Emits scores + tri for kt."""
                    par = kt % 2
                    for h in range(2):
                        for ci, qc in enumerate(qcs):
                            sl = slot(s, kt, qc)
                            nc.tensor.matmul(
                                st[:, h, ci, :],
                                ktb[64 * h:64 * (h + 1), kt:kt + 2, j, :],
                                qz[64 * h:64 * (h + 1), :, j, NCH * qc:NCH * (qc + 1)],
                                start=(ci == 0), stop=(ci == len(qcs) - 1),
                                perf_mode=DR, tile_position=(64 * h, 0))
                            # diagonal triangle: -256 into the masked region
                            nc.tensor.matmul(
                                st[:, h, ci, P * par:P * (par + 1)],
                                negI[:, :], mrhs[:, sl, :],
                                start=False, stop=False, skip_group_check=True)
                            if par == 1:
                                # odd diagonal block: first q-half fully dead
                                nc.tensor.matmul(
                                    st[:, h, ci, 0:P],
                                    negI[:, :], mkill[:, sl, :],
                                    start=False, stop=False, skip_group_check=True)
                            if s == 0 and qc == 0 and kt >= 2:
                                # beyond-diagonal chunk-0 tile: fully dead for
                                # ranks with 2p+2 <= kt (data-driven via dkill)
                                nc.tensor.matmul(
                                    st[:, h, 0, :],
                                    negI[:, :], dkill[:, kt - 2, h, :],
                                    start=False, stop=False, skip_group_check=True)

                def exp_block(j, s, kt, st, et, kslot, qcs):
                    if len(qcs) == 2:
                        # merged both-chunk exp; dead tiles already killed on PE
                        nc.scalar.activation(
                            out=et[:, kslot, :, :, :], in_=st[:, :, :, :],
                            func=AF.Exp, scale=SCALE, bias=expb_t)
                    else:
                        sl = slot(s, kt, qcs[0])
                        nc.scalar.activation(
                            out=et[:, kslot, :, 0, :], in_=st[:, :, 0, :],
                            func=AF.Exp, scale=SCALE, bias=be[:, sl:sl + 1])

                def av_block(j, pair, et, vb, ya, cis, starts, stops):
                    for h in range(2):
                        for ei, (ci, start, stop) in enumerate(zip(cis, starts, stops)):
                            nc.tensor.matmul(
                                ya[h][:, ci, :],
                                vb[:, 2 * pair:2 * pair + 2, 2 * j + h, :],
                                et[:, :, h, ei, :],
                                start=start, stop=stop, perf_mode=DR)

                # ---- load both phases up front ----
                ktbA, vbA = load_kv(0)
                ktbB, vbB = load_kv(1)
                with tc.tile_pool(name="at_ps0", bufs=1, space="PSUM") as at_ps:
                    for j in range(8):
                        ya = [at_ps.tile([D + 1, 2, NCH], F32, name=f"ya{h}", tag=f"ya{h}", bufs=1)
                              for h in range(2)]
                        # ---- phase A: keys 0:1024, both chunks ----
                        for pair in range(4):
                            et = sm.tile([P, 2, 2, 2, NCH], FP8D, name="et", tag="et", bufs=2)
                            for kslot in range(2):
                                kt = 2 * pair + kslot
                                st = at_ps.tile([P, 2, 2, NCH], F32, name="st", tag="st",
                                                bufs=2)
                                score_block(j, 0, kt, st, ktbA, (0, 1))
                                exp_block(j, 0, kt, st, et, kslot, (0, 1))
                            av_block(j, pair, et, vbA, ya, (0, 1),
                                     starts=(pair == 0, False),
                                     stops=(False, False))
                        for h in range(2):
                            div_write(h, j, 0, ya[h][0:D, 0, :], ya[h][D:D + 1, 0, :])
                        # ---- phase B: keys 1024:2048, chunk 1 only ----
                        for pair in range(4):
                            etb = sm.tile([P, 2, 2, 1, NCH], FP8D, name="etb", tag="etb", bufs=2)
                            for kslot in range(2):
                                kt = 2 * pair + kslot
                                stb = at_ps.tile([P, 2, 2, NCH], F32, name="st", tag="st",
                                                 bufs=2)
                                score_block(j, 1, kt, stb, ktbB, (1,))
                                exp_block(j, 1, kt, stb, etb, kslot, (1,))
                            av_block(j, pair, etb, vbB, ya, (1,),
                                     starts=(False,), stops=(pair == 3,))
                        for h in range(2):
                            div_write(h, j, 1, ya[h][0:D, 1, :], ya[h][D:D + 1, 1, :])

                proj([0, 1, 2, 3])

        # ================= LN2 + MLP =================
        with tc.tile_pool(name="mlpp", bufs=1) as mp, \
             tc.tile_pool(name="wmlp", bufs=3) as wmp:
            ln2 = layer_norm(x2, mp, "ln2", dt=BF16)
            xln2T = transpose_to(ln2, mp, "xln2T", dt=FC1D, idn=ident_bf)
            for t in range(4):
                nc.gpsimd.tensor_tensor(out=x2[:, t, :], in0=x2[:, t, :], in1=b2_bc, op=ALU.add)

            h_sb = mp.tile([P, 32, 512], FC2D)
            for half in range(2):
                with tc.tile_pool(name=f"mlp_ps{half}", bufs=1, space="PSUM") as mlp_ps:
                    ops = [mlp_ps.tile([P, 512], F32, name=f"ops{t}", tag=f"ops{t}", bufs=1)
                           for t in range(4)]
                    for m in range(32):
                        if half == 0:
                            if m % 4 == 0:
                                wfc = wmp.tile([P, 8, 512], FC1D, name="wfc", tag="wfc")
                                nc.sync.dma_start(out=wfc,
                                                    in_=wfc_d[:, 512 * (m // 4):512 * (m // 4 + 1)]
                                                    .rearrange("(kc kp) n -> kp kc n", kp=P))
                            mo = P * (m % 4)
                            fps = mlp_ps.tile([P, 512], F32, name="fps", tag="fps", bufs=4)
                            if FP8_FC1:
                                for k in range(4):
                                    for hh in range(2):
                                        nc.tensor.matmul(fps[:, 256 * hh:256 * (hh + 1)],
                                                         wfc[:, 2 * k:2 * k + 2, mo:mo + P],
                                                         xln2T[:, 2 * k:2 * k + 2, 256 * hh:256 * (hh + 1)],
                                                         start=(k == 0 and hh == 0),
                                                         stop=(k == 3 and hh == 1), perf_mode=DR)
                            else:
                                for k in range(8):
                                    nc.tensor.matmul(fps[:, :], wfc[:, k, mo:mo + P], xln2T[:, k, :],
                                                     start=(k == 0), stop=(k == 7))
                            nc.scalar.activation(out=h_sb[:, m, :], in_=fps[:, :], func=AF.Gelu,
                                                 bias=bfc_sb[:, m:m + 1], scale=1.0)
                        if m % 4 == 0:
                            w2 = wmp.tile([P, 4, 512], FC2D, name="w2", tag="w2", bufs=3)
                            nc.sync.dma_start(out=w2, in_=wfc2_d[P * m:P * (m + 4),
                                                               512 * half:512 * (half + 1)]
                                                .rearrange("(mc mp) n -> mp mc n", mp=P))
                        if FP8_FC2:
                            if m % 2 == 0:
                                for t in range(4):
                                    for n in range(2):
                                        nc.tensor.matmul(
                                            ops[t][:, 256 * n:256 * (n + 1)],
                                            h_sb[:, m:m + 2, P * t:P * (t + 1)],
                                            w2[:, (m % 4):(m % 4) + 2, 256 * n:256 * (n + 1)],
                                            start=(m == 0 and n == 0),
                                            stop=(m == 30 and n == 1), perf_mode=DR)
                        else:
                            for t in range(4):
                                nc.tensor.matmul(ops[t][:, :], h_sb[:, m, P * t:P * (t + 1)],
                                                 w2[:, m % 4, :], start=(m == 0), stop=(m == 31))
                    for t in range(4):
                        nc.vector.tensor_tensor(out=x2[:, t, 512 * half:512 * (half + 1)],
                                                in0=ops[t][:, :],
                                                in1=x2[:, t, 512 * half:512 * (half + 1)], op=ALU.add)
                        if half == 1:
                            nc.sync.dma_start(out=out_ext[P * t:P * (t + 1), :], in_=x2[:, t, :])

    nc.finalize()
    return nc


def _get_nc():
    if "nc" not in _CACHE:
        _CACHE["nc"] = _build()
    return _CACHE["nc"]


def _prep(**inputs):
    f = lambda a: np.asarray(a, dtype=np.float32)
    x = f(inputs["x"])
    ln1_g, ln1_b = f(inputs["ln1_g"]), f(inputs["ln1_b"])
    ln2_g, ln2_b = f(inputs["ln2_g"]), f(inputs["ln2_b"])
    W_attn, b_attn = f(inputs["W_attn"]), f(inputs["b_attn"])
    W_o, b_o = f(inputs["W_o"]), f(inputs["b_o"])
    W_fc, b_fc = f(inputs["W_fc"]), f(inputs["b_fc"])
    W_fc2, b_fc2 = f(inputs["W_fc2"]), f(inputs["b_fc2"])

    # fold LN affine params into the next matmul
    W_attn_e = ln1_g[:, None] * W_attn
    b_attn_e = b_attn + ln1_b @ W_attn
    W_fc_e = ln2_g[:, None] * W_fc
    b_fc_e = b_fc + ln2_b @ W_fc
    # V bias contributes a constant through attention: fold b_v @ W_o into
    # the residual bias (K bias is softmax-invariant and dropped).
    rb = b_o + b_attn_e[2 * C:3 * C] @ W_o

    fc1d = FP8 if FP8_FC1 else ml_dtypes.bfloat16
    fc2d = FP8 if FP8_FC2 else ml_dtypes.bfloat16

    in_maps = []
    for r in range(N_CORES):
        b, p = divmod(r, 4)
        c0, c1 = p, 7 - p
        xs = np.concatenate([x[b, NCH * c0:NCH * (c0 + 1)],
                             x[b, NCH * c1:NCH * (c1 + 1)]], axis=0)
        in_maps.append({
            "x": np.ascontiguousarray(xs),
            "qbase": np.array([[NCH * c0, NCH * c1]], dtype=np.float32),
            "wq": W_attn_e[:, 0:C].astype(FP8),
            "wk": W_attn_e[:, C:2 * C].astype(FP8),
            "wv": W_attn_e[:, 2 * C:3 * C].astype(FP8),
            "bq": b_attn_e[0:C],
            "wo": W_o.astype(FP8), "rb": rb,
            "w_fc": W_fc_e.astype(fc1d), "b_fc": b_fc_e,
            "w_fc2": W_fc2.astype(fc2d), "b_fc2": b_fc2,
        })

    def assemble(results):
        out = np.empty((B, T, C), dtype=np.float32)
        for r in range(N_CORES):
            b, p = divmod(r, 4)
            c0, c1 = p, 7 - p
            o = results[r]["out"]
            out[b, NCH * c0:NCH * (c0 + 1)] = o[0:NCH]
            out[b, NCH * c1:NCH * (c1 + 1)] = o[NCH:TOK]
        return out

    return in_maps, assemble


def kernel(**inputs):
    from concourse.bass_utils import run_bass_kernel_spmd

    in_maps, assemble = _prep(**inputs)
    res = run_bass_kernel_spmd(_get_nc(), in_maps, list(range(N_CORES)))
    return assemble(res.results)


# revision 21
# speedup vs baseline: 1.6259x; 1.0112x over previous
"""Transformer block (pre-LN causal MHA + GELU MLP) on 8 trn2 NeuronCores.

Sharding: core r handles batch b=r//4, group position p=r%4, owning token
chunks {p, 7-p} of eight 256-token chunks (causally balanced zigzag).
Sequence-parallel everywhere except attention: K^T and V for the full batch
are exchanged via fp8 AllGathers inside each 4-core batch group.

All heavy matmuls run in fp8e4 with DoubleRow perf mode (2 contraction
k-tiles per instruction at 0.5 cycles/row): QKV projections, attention
scores (K=64 with a zeroed second subtile on the Q side), attention*V
(key-tile pairs), output projection, and the MLP (precision tier
selectable per matmul via the FP8_* flags).

Masking is done on the PE + Act engines instead of element-wise DVE
multiplies: fully-masked (key-block, chunk) tiles get exp bias -30 from a
data-driven per-tile bias table (exp underflows to 0 in fp8), and the two
diagonal key-blocks per chunk get -256 added to the masked triangle via a
single extra matmul (lhsT=-256*I, rhs=triangle indicator built from qbase)
before the exp. Scores carry no 1/sqrt(d) or softmax-max handling: the
scale (0.125) and a -4*ln2 range shift are folded into the exp activation
(exp output ~ exp(s)/16 stays within fp8e4 range; the shift cancels in the
softmax division).

Bias handling: K bias is dropped (softmax is invariant to per-query score
shifts), V bias is folded into the residual bias on the host
(b_o + b_v @ W_o), Q bias is applied on the PSUM->SBUF copy, fc bias rides
the GELU activation, fc2 bias is pre-added to the residual.

LN rsqrt = exp(-0.5*ln(var+eps)) so LN1/attention/LN2 share one activation
table (natural_log_exp) and only the MLP's gelu forces a table switch.

Self-contained: hardcodes B=2, T=2048, C=1024, H=16, D=64, hidden=4096.
"""
import sys

if "/opt/trn_rl_repo" not in sys.path:
    sys.path.insert(0, "/opt/trn_rl_repo")

import numpy as np
import ml_dtypes

B, T, C, H = 2, 2048, 1024, 16
D = C // H            # 64
MH = 4 * C            # 4096 mlp hidden
EPS = 1e-5
P = 128
TOK = 512             # tokens per core
NCH = 256             # tokens per chunk
N_CORES = 8
EXPB = -2.7725887     # -4*ln2: exp emits exp(s)/16
SCALE = 0.125         # 1/sqrt(D)

# precision tiers (fp8 DoubleRow vs bf16) — tuned empirically
FP8_FC1 = True
FP8_FC2 = True

FP8 = ml_dtypes.float8_e4m3

_CACHE: dict = {}


def _build(mock_cc=False):
    import concourse.tile as tile
    from concourse import bacc, mybir
    from concourse.masks import make_identity
    from contextlib import ExitStack

    F32 = mybir.dt.float32
    BF16 = mybir.dt.bfloat16
    FP8D = mybir.dt.float8e4
    I32 = mybir.dt.int32
    AF = mybir.ActivationFunctionType
    ALU = mybir.AluOpType
    DR = mybir.MatmulPerfMode.DoubleRow

    FC1D = FP8D if FP8_FC1 else BF16
    FC2D = FP8D if FP8_FC2 else BF16

    nc = bacc.Bacc()

    # ---------------- I/O ----------------
    x_in = nc.declare_dram_parameter("x", [TOK, C], F32, isOutput=False)
    qbase_in = nc.declare_dram_parameter("qbase", [1, 2], F32, isOutput=False)
    wq_d = nc.declare_dram_parameter("wq", [C, C], FP8D, isOutput=False)
    wk_d = nc.declare_dram_parameter("wk", [C, C], FP8D, isOutput=False)
    wv_d = nc.declare_dram_parameter("wv", [C, C], FP8D, isOutput=False)
    bq_d = nc.declare_dram_parameter("bq", [C], F32, isOutput=False)
    wo_d = nc.declare_dram_parameter("wo", [C, C], FP8D, isOutput=False)
    rb_d = nc.declare_dram_parameter("rb", [C], F32, isOutput=False)
    wfc_d = nc.declare_dram_parameter("w_fc", [C, MH], FC1D, isOutput=False)
    bfc_d = nc.declare_dram_parameter("b_fc", [MH], F32, isOutput=False)
    wfc2_d = nc.declare_dram_parameter("w_fc2", [MH, C], FC2D, isOutput=False)
    bfc2_d = nc.declare_dram_parameter("b_fc2", [C], F32, isOutput=False)
    out_ext = nc.declare_dram_parameter("out", [TOK, C], F32, isOutput=True)

    # internal DRAM for the collectives (half s=0: keys 0:1024, s=1: 1024:2048)
    kt_in = [nc.dram_tensor(f"kt_in_{s}", [C, NCH], FP8D) for s in range(2)]
    v_in = [nc.dram_tensor(f"v_in_{s}", [NCH, C], FP8D) for s in range(2)]
    kt_all = [nc.dram_tensor(f"kt_all_{s}", [4 * C, NCH], FP8D) for s in range(2)]
    v_all = [nc.dram_tensor(f"v_all_{s}", [4 * NCH, C], FP8D) for s in range(2)]
    stash_d = nc.dram_tensor("stash_d", [16, D + 1, NCH], F32)
    RG = [[0, 1, 2, 3], [4, 5, 6, 7]]

    with tile.TileContext(nc) as tc, ExitStack() as ctx:
        # ---------- pools: outer (whole kernel) ----------
        const = ctx.enter_context(tc.tile_pool(name="const", bufs=1))
        outer = ctx.enter_context(tc.tile_pool(name="outer", bufs=1))
        sm = ctx.enter_context(tc.tile_pool(name="sm", bufs=2))
        wmlp_outer = ctx.enter_context(tc.tile_pool(name="wmlp", bufs=3))

        # ---------- constants ----------
        ident = const.tile([P, P], F32)
        make_identity(nc, ident)
        ident_bf = const.tile([P, P], BF16)
        nc.vector.tensor_copy(out=ident_bf, in_=ident)
        eps_t = const.tile([P, 1], F32)
        nc.vector.memset(eps_t, EPS)
        expb_t = const.tile([P, 1], F32)
        nc.vector.memset(expb_t, EXPB)
        ones128 = const.tile([P, P], F32)
        nc.vector.memset(ones128, 1.0)
        # -128 * I in fp8 (tri-mask stationary operand; exp(s/8-16) -> 0 in fp8)
        negI = const.tile([P, P], FP8D)
        negI_f = const.tile([P, P], F32)
        nc.vector.tensor_scalar(out=negI_f, in0=ident, scalar1=-128.0, scalar2=None,
                                op0=ALU.mult)
        nc.vector.tensor_copy(out=negI, in_=negI_f)

        bq_sb = const.tile([P, 8], F32)     # q bias -> [128, 8]
        nc.sync.dma_start(out=bq_sb, in_=bq_d[0:C].rearrange("(f p) -> p f", p=P))
        bfc_sb = const.tile([P, 32], F32)
        nc.sync.dma_start(out=bfc_sb, in_=bfc_d[:].rearrange("(f p) -> p f", p=P))
        rb_bc = const.tile([P, C], F32)     # residual bias (b_o + b_v@W_o) bcast
        nc.sync.dma_start(out=rb_bc, in_=rb_d[:].rearrange("(a c) -> a c", a=1).to_broadcast((P, C)))
        b2_bc = const.tile([P, C], F32)
        nc.sync.dma_start(out=b2_bc, in_=bfc2_d[:].rearrange("(a c) -> a c", a=1).to_broadcast((P, C)))

        # qbase + iotas for mask tables
        qbase_sb = const.tile([1, 2], F32)
        nc.sync.dma_start(out=qbase_sb, in_=qbase_in[:, :])
        kidx_i = const.tile([P, 1], I32)
        nc.gpsimd.iota(kidx_i, pattern=[[0, 1]], base=0, channel_multiplier=1)
        kidx_f = const.tile([P, 1], F32)
        nc.vector.tensor_copy(out=kidx_f, in_=kidx_i)
        qio_i = const.tile([1, P], I32)
        nc.gpsimd.iota(qio_i, pattern=[[1, P]], base=0, channel_multiplier=0)
        qio_f = const.tile([1, P], F32)
        nc.vector.tensor_copy(out=qio_f, in_=qio_i)
        # TRI[k, q] = 1 if q < k else 0  (masked region of an aligned 128-diag)
        qio_bc = const.tile([P, P], F32)
        nc.gpsimd.partition_broadcast(qio_bc, qio_f)
        tri_f = const.tile([P, P], F32)
        nc.vector.tensor_scalar(out=tri_f, in0=qio_bc, scalar1=kidx_f, scalar2=None,
                                op0=ALU.is_lt)

        # ---- per-(phase, kt, chunk) exp bias table: alive -> EXPB, dead -> -30
        # slot order: (s, kt, c) -> 32 slots (s in 0..1, kt 0..7, c 0..1)
        kb_i = const.tile([1, 32], I32)
        nc.gpsimd.iota(kb_i, pattern=[[1024, 2], [128, 8], [0, 2]], base=0,
                       channel_multiplier=0)
        kb_f = const.tile([1, 32], F32)
        nc.vector.tensor_copy(out=kb_f, in_=kb_i)
        csel_i = const.tile([1, 32], I32)   # 0,1,0,1,... chunk selector
        nc.gpsimd.iota(csel_i, pattern=[[0, 2], [0, 8], [1, 2]], base=0,
                       channel_multiplier=0)
        csel_f = const.tile([1, 32], F32)
        nc.vector.tensor_copy(out=csel_f, in_=csel_i)
        # qb_slot = qbase[c0] + csel*(qbase[c1]-qbase[c0])
        qdiff = const.tile([1, 1], F32)
        nc.vector.tensor_scalar(out=qdiff, in0=qbase_sb[0:1, 1:2],
                                scalar1=qbase_sb[0:1, 0:1], scalar2=None,
                                op0=ALU.subtract)
        qb_slot = const.tile([1, 32], F32)
        nc.vector.tensor_scalar(out=qb_slot, in0=csel_f, scalar1=qdiff,
                                scalar2=qbase_sb[0:1, 0:1], op0=ALU.mult, op1=ALU.add)
        # alive = (qb_slot + 255 >= kb)  <=>  qb_slot - kb >= -255
        alive = const.tile([1, 32], F32)
        nc.vector.tensor_tensor(out=alive, in0=qb_slot, in1=kb_f, op=ALU.subtract)
        nc.vector.tensor_scalar(out=alive, in0=alive, scalar1=-255.0, scalar2=None,
                                op0=ALU.is_ge)
        be_row = const.tile([1, 32], F32)   # -30 + alive*(30+EXPB)
        nc.vector.tensor_scalar(out=be_row, in0=alive, scalar1=30.0 + EXPB,
                                scalar2=-30.0, op0=ALU.mult, op1=ALU.add)
        be = const.tile([P, 32], F32)
        nc.gpsimd.partition_broadcast(be, be_row)

        # ---- tri-mask rhs table: mrhs[:, slot, :] = TRI * diag(slot)
        # diag(slot) = 1 iff kb[slot] == qb_slot + 128*parity(kt)
        par_i = const.tile([1, 32], I32)
        nc.gpsimd.iota(par_i, pattern=[[0, 2], [0, 4], [128, 2], [0, 2]], base=0,
                       channel_multiplier=0)   # (s, ktpair, par, c) -> 128*(kt%2)
        par_f = const.tile([1, 32], F32)
        nc.vector.tensor_copy(out=par_f, in_=par_i)
        dfl = const.tile([1, 32], F32)
        nc.vector.tensor_tensor(out=dfl, in0=kb_f, in1=par_f, op=ALU.subtract)
        nc.vector.tensor_tensor(out=dfl, in0=dfl, in1=qb_slot, op=ALU.is_equal)
        dflb = const.tile([P, 32], F32)
        nc.gpsimd.partition_broadcast(dflb, dfl)
        mrhs = const.tile([P, 32, P], FP8D)
        for sl in range(32):
            nc.vector.tensor_scalar(out=mrhs[:, sl, :], in0=tri_f,
                                    scalar1=dflb[:, sl:sl + 1], scalar2=None,
                                    op0=ALU.mult)
        # full-kill pattern for the odd diagonal block's dead first q-half
        mkill = const.tile([P, 32, P], FP8D)
        for sl in range(32):
            if (sl // 2) % 2 == 1:   # odd kt slots only
                nc.vector.tensor_scalar(out=mkill[:, sl, :], in0=ones128,
                                        scalar1=dflb[:, sl:sl + 1], scalar2=None,
                                        op0=ALU.mult)
        # dead = 1 - alive, broadcast per slot
        deadb = const.tile([P, 32], F32)
        nc.vector.tensor_scalar(out=deadb, in0=dflb, scalar1=0.0, scalar2=None,
                                op0=ALU.mult)   # placeholder shape; overwritten
        dead_row = const.tile([1, 32], F32)
        nc.vector.tensor_scalar(out=dead_row, in0=alive, scalar1=-1.0,
                                scalar2=1.0, op0=ALU.mult, op1=ALU.add)
        nc.gpsimd.partition_broadcast(deadb, dead_row)
        ones2 = const.tile([P, 2, NCH], F32)
        nc.vector.memset(ones2, 1.0)
        dkill = const.tile([P, 6, 2, NCH], FP8D)
        for i in range(6):
            sl = 2 * (i + 2)   # slot(0, kt=i+2, c=0)
            nc.vector.tensor_scalar(out=dkill[:, i, :, :], in0=ones2,
                                    scalar1=deadb[:, sl:sl + 1], scalar2=None,
                                    op0=ALU.mult)

        def slot(s, kt, c):
            return s * 16 + kt * 2 + c

        # ---------- helpers ----------
        def layer_norm(src, dst_pool, tag, dt=BF16):
            # per-t sqrt (Sqrt table shared across all 4 calls; Exp/Gelu load later)
            ln = dst_pool.tile([P, 4, C], dt, name=tag, tag=tag)
            for t in range(4):
                stats = sm.tile([P, 2, 6], F32, name="lnstats", tag="lnstats")
                nc.vector.bn_stats(out=stats[:, 0, :], in_=src[:, t, 0:512])
                nc.vector.bn_stats(out=stats[:, 1, :], in_=src[:, t, 512:1024])
                mv = sm.tile([P, 2], F32, name="lnmv", tag="lnmv")
                nc.vector.bn_aggr(out=mv, in_=stats)
                rstd = sm.tile([P, 1], F32, name="lnrstd", tag="lnrstd")
                nc.scalar.activation(out=rstd, in_=mv[:, 1:2], func=AF.Sqrt,
                                     bias=eps_t, scale=1.0)
                nc.vector.reciprocal(out=rstd, in_=rstd)
                nmu = sm.tile([P, 1], F32, name="lnnmu", tag="lnnmu")
                nc.vector.tensor_scalar(out=nmu, in0=mv[:, 0:1], scalar1=rstd,
                                        scalar2=-1.0, op0=ALU.mult, op1=ALU.mult)
                nc.scalar.activation(out=ln[:, t, :], in_=src[:, t, :],
                                     func=AF.Identity, bias=nmu, scale=rstd)
            return ln

        def transpose_to(lnt, dst_pool, dst_tag, dt, idn):
            xt = dst_pool.tile([P, 8, TOK], dt, name=dst_tag, tag=dst_tag)
            with tc.tile_pool(name="tp_ps", bufs=2, space="PSUM") as tp_ps:
                for t in range(4):
                    for f in range(8):
                        pt = tp_ps.tile([P, P], lnt.dtype, name="tpt", tag="tpt",
                                        padded_shape=[P, 2 * P])
                        nc.tensor.transpose(pt[:, :], lnt[:, t, P * f:P * (f + 1)], idn)
                        eng = nc.vector if f % 2 == 0 else nc.scalar
                        if f % 2 == 0:
                            nc.vector.tensor_copy(out=xt[:, f, P * t:P * (t + 1)], in_=pt[:, :])
                        else:
                            nc.scalar.activation(out=xt[:, f, P * t:P * (t + 1)], in_=pt[:, :],
                                                 func=AF.Copy)
            return xt

        x2 = outer.tile([P, 4, C], F32)
        yT = outer.tile([P, 8, TOK], FP8D)

        with tc.tile_pool(name="mid", bufs=1) as mid:
            x_sb = mid.tile([P, 4, C], F32)
            for t in range(4):
                (nc.sync if t % 2 == 0 else nc.scalar).dma_start(
                    out=x_sb[:, t, :], in_=x_in[P * t:P * (t + 1), :])
            # qz: [part, sub(2), j, tok]  sub1 = zeros (DoubleRow zero-subtile)
            qz = mid.tile([P, 2, 8, TOK], FP8D)
            nc.gpsimd.memset(qz[:, 1, :, :], 0.0)

            # ================= qkv =================
            with tc.tile_pool(name="qkvp", bufs=1) as qp, \
                 tc.tile_pool(name="wqkv", bufs=2) as wp:
                ln1 = layer_norm(x_sb, qp, "ln")
                xT8 = transpose_to(ln1, qp, "xT8", FP8D, ident_bf)
                qkv_ps_cm = tc.tile_pool(name="qkv_ps", bufs=3, space="PSUM")
                qkv_ps = qkv_ps_cm.__enter__()

                wk_sb = wp.tile([P, 8, C], FP8D, name="wk", tag="wk")
                nc.scalar.dma_start(out=wk_sb, in_=wk_d[:, :].rearrange("(kc kp) n -> kp kc n", kp=P))
                # K^T feature tiles -> kt_in halves (K bias dropped: softmax-invariant)
                for f in range(8):
                    fo = P * f
                    ps = qkv_ps.tile([P, TOK], F32, name="kps", tag="qkvps")
                    for k in range(4):
                        for hh in range(2):
                            nc.tensor.matmul(ps[:, TOK // 2 * hh:TOK // 2 * (hh + 1)],
                                             wk_sb[:, 2 * k:2 * k + 2, fo:fo + P],
                                             xT8[:, 2 * k:2 * k + 2, 256 * hh:256 * (hh + 1)],
                                             start=(k == 0 and hh == 0),
                                             stop=(k == 3 and hh == 1), perf_mode=DR)
                    kt_sb = sm.tile([P, TOK], FP8D, name="kt_sb", tag="kt_sb", bufs=2)
                    nc.scalar.activation(out=kt_sb, in_=ps[:, :], func=AF.Copy)
                    for s in range(2):
                        nc.sync.dma_start(out=kt_in[s][P * f:P * (f + 1), :],
                                          in_=kt_sb[:, NCH * s:NCH * (s + 1)])
                # kt gathers fire as soon as K^T is written
                for s in range(2):
                    if mock_cc:
                        (nc.scalar if s == 0 else nc.sync).dma_start(
                            out=kt_all[s][0:C, :], in_=kt_in[s][:, :])
                    else:
                        nc.gpsimd.collective_compute("AllGather", ALU.bypass,
                                                     ins=[kt_in[s][:, :]], outs=[kt_all[s][:, :]],
                                                     replica_groups=RG)
                # V token tiles -> v_in halves (V bias folded into residual bias)
                wv_sb = wp.tile([P, 8, C], FP8D, name="wv", tag="wk")
                nc.sync.dma_start(out=wv_sb, in_=wv_d[:, :].rearrange("(kc kp) n -> kp kc n", kp=P))
                for t in range(4):
                    ps = qkv_ps.tile([P, C], F32, name="vps", tag="vps", bufs=2)
                    for k in range(4):
                        for n in range(4):
                            nc.tensor.matmul(ps[:, NCH * n:NCH * (n + 1)],
                                             xT8[:, 2 * k:2 * k + 2, P * t:P * (t + 1)],
                                             wv_sb[:, 2 * k:2 * k + 2, NCH * n:NCH * (n + 1)],
                                             start=(k == 0 and n % 2 == 0),
                                             stop=(k == 3 and n % 2 == 1), perf_mode=DR)
                    v_sb = sm.tile([P, C], FP8D, name="v_sb", tag="v_sb")
                    nc.scalar.activation(out=v_sb, in_=ps[:, :], func=AF.Copy)
                    sh, row = divmod(t, 2)
                    nc.sync.dma_start(out=v_in[sh][P * row:P * (row + 1), :], in_=v_sb)
                    if row == 1:
                        if mock_cc:
                            nc.gpsimd.dma_start(
                                out=v_all[sh][0:NCH, :], in_=v_in[sh][:, :])
                        else:
                            nc.gpsimd.collective_compute("AllGather", ALU.bypass,
                                                         ins=[v_in[sh][:, :]], outs=[v_all[sh][:, :]],
                                                         replica_groups=RG)

                # Q^T feature tiles (stay local); bias on copy, scale folded in exp
                wq_sb = wp.tile([P, 8, C], FP8D, name="wq", tag="wk")
                nc.sync.dma_start(out=wq_sb, in_=wq_d[:, :].rearrange("(kc kp) n -> kp kc n", kp=P))
                for f in range(8):
                    fo = P * f
                    ps = qkv_ps.tile([P, TOK], F32, name="qps", tag="qkvps")
                    for k in range(4):
                        for hh in range(2):
                            nc.tensor.matmul(ps[:, TOK // 2 * hh:TOK // 2 * (hh + 1)],
                                             wq_sb[:, 2 * k:2 * k + 2, fo:fo + P],
                                             xT8[:, 2 * k:2 * k + 2, 256 * hh:256 * (hh + 1)],
                                             start=(k == 0 and hh == 0),
                                             stop=(k == 3 and hh == 1), perf_mode=DR)
                    nc.vector.tensor_scalar(out=qz[:, 0, f, :], in0=ps[:, :],
                                            scalar1=bq_sb[:, f:f + 1], scalar2=None,
                                            op0=ALU.add)
                qkv_ps_cm.__exit__(None, None, None)

            # ============ attention (+ proj overlapped into phase B) ============
            with tc.tile_pool(name="attp", bufs=1) as ap, \
                 tc.tile_pool(name="projp", bufs=1) as pp, \
                 tc.tile_pool(name="pr_ps", bufs=2, space="PSUM") as pr_ps:
                wo_sb = pp.tile([P, 8, C], FP8D)
                nc.sync.dma_start(out=wo_sb, in_=wo_d[:, :].rearrange("(kc kp) n -> kp kc n", kp=P))
                for t in range(4):
                    nc.gpsimd.tensor_tensor(out=x_sb[:, t, :], in0=x_sb[:, t, :], in1=rb_bc, op=ALU.add)

                def load_kv(s):
                    # ktb: [part(2h d), kt-slot(8+1 pad), j, keys]
                    ktb = ap.tile([P, 9, 8, P], FP8D, name="ktb", tag="ktb", bufs=2)
                    nc.gpsimd.memset(ktb[:, 8, :, :], 0.0)
                    # vb: [part(key), kt-slot, hh, D+1]
                    vb = ap.tile([P, 8, 16, D + 1], FP8D, name="vb", tag="vb", bufs=2)
                    nc.vector.tensor_copy(out=vb[:, :, :, D:D + 1],
                                          in_=ones128.rearrange("p (a b) -> p a b", a=8)[:, :, 0:16])
                    for r in range(4):
                        nc.sync.dma_start(
                            out=ktb[:, 2 * r:2 * r + 2, :, :],
                            in_=kt_all[s][C * r:C * (r + 1), :].rearrange(
                                "(j p) (kb kc) -> p kb j kc", p=P, kb=2))
                        for sub in range(2):
                            nc.sync.dma_start(
                                out=vb[:, 2 * r + sub, :, 0:D],
                                in_=v_all[s][NCH * r + P * sub:NCH * r + P * (sub + 1), :]
                                        .rearrange("p (h d) -> p h d", h=H))
                    return ktb, vb

                def div_write(h, j, qc, ysrc, rsrc):
                    recip = sm.tile([1, NCH], F32, name=f"rc{h}", tag=f"rc{h}")
                    nc.vector.reciprocal(out=recip, in_=rsrc)
                    rb = sm.tile([D, NCH], F32, name=f"rb{h}", tag=f"rb{h}")
                    nc.gpsimd.partition_broadcast(rb, recip)
                    nc.vector.tensor_tensor(out=yT[64 * h:64 * (h + 1), j, NCH * qc:NCH * (qc + 1)],
                                            in0=ysrc, in1=rb, op=ALU.mult)

                def proj(trange):
                    for t in trange:
                        for nn in range(2):
                            ps = pr_ps.tile([P, 512], F32, name="prps", tag="prps")
                            for k in range(4):
                                for n2 in range(2):
                                    nc.tensor.matmul(
                                        ps[:, NCH * n2:NCH * (n2 + 1)],
                                        yT[:, 2 * k:2 * k + 2, P * t:P * (t + 1)],
                                        wo_sb[:, 2 * k:2 * k + 2, 512 * nn + NCH * n2:512 * nn + NCH * (n2 + 1)],
                                        start=(k == 0 and n2 == 0),
                                        stop=(k == 3 and n2 == 1), perf_mode=DR)
                            nc.vector.tensor_tensor(out=x2[:, t, 512 * nn:512 * (nn + 1)], in0=ps[:, :],
                                                    in1=x_sb[:, t, 512 * nn:512 * (nn + 1)], op=ALU.add)

                def score_block(j, s, kt, st, ktb, qcs):
                    """st: psum [P, 2h, len(qcs), NCH]. Emits scores + tri for kt."""
                    par = kt % 2
                    for h in range(2):
                        for ci, qc in enumerate(qcs):
                            sl = slot(s, kt, qc)
                            nc.tensor.matmul(
                                st[:, h, ci, :],
                                ktb[64 * h:64 * (h + 1), kt:kt + 2, j, :],
                                qz[64 * h:64 * (h + 1), :, j, NCH * qc:NCH * (qc + 1)],
                                start=(ci == 0), stop=(ci == len(qcs) - 1),
                                perf_mode=DR, tile_position=(64 * h, 0))
                            # diagonal triangle: -256 into the masked region
                            nc.tensor.matmul(
                                st[:, h, ci, P * par:P * (par + 1)],
                                negI[:, :], mrhs[:, sl, :],
                                start=False, stop=False, skip_group_check=True)
                            if par == 1:
                                # odd diagonal block: first q-half fully dead
                                nc.tensor.matmul(
                                    st[:, h, ci, 0:P],
                                    negI[:, :], mkill[:, sl, :],
                                    start=False, stop=False, skip_group_check=True)
                            if s == 0 and qc == 0 and kt >= 2:
                                # beyond-diagonal chunk-0 tile: fully dead for
                                # ranks with 2p+2 <= kt (data-driven via dkill)
                                nc.tensor.matmul(
                                    st[:, h, 0, :],
                                    negI[:, :], dkill[:, kt - 2, h, :],
                                    start=False, stop=False, skip_group_check=True)

                def exp_block(j, s, kt, st, et, kslot, qcs):
                    if len(qcs) == 2:
                        # merged both-chunk exp; dead tiles already killed on PE
                        nc.scalar.activation(
                            out=et[:, kslot, :, :, :], in_=st[:, :, :, :],
                            func=AF.Exp, scale=SCALE, bias=expb_t)
                    else:
                        sl = slot(s, kt, qcs[0])
                        nc.scalar.activation(
                            out=et[:, kslot, :, 0, :], in_=st[:, :, 0, :],
                            func=AF.Exp, scale=SCALE, bias=be[:, sl:sl + 1])

                def av_block(j, pair, et, vb, ya, cis, starts, stops):
                    for h in range(2):
                        for ei, (ci, start, stop) in enumerate(zip(cis, starts, stops)):
                            nc.tensor.matmul(
                                ya[h][:, ci, :],
                                vb[:, 2 * pair:2 * pair + 2, 2 * j + h, :],
                                et[:, :, h, ei, :],
                                start=start, stop=stop, perf_mode=DR)

                # ---- load both phases up front ----
                ktbA, vbA = load_kv(0)
                ktbB, vbB = load_kv(1)
                with tc.tile_pool(name="at_ps0", bufs=1, space="PSUM") as at_ps:
                    for j in range(8):
                        ya = [at_ps.tile([D + 1, 2, NCH], F32, name=f"ya{h}", tag=f"ya{h}", bufs=1)
                              for h in range(2)]
                        # ---- phase A: keys 0:1024, both chunks ----
                        for pair in range(4):
                            et = sm.tile([P, 2, 2, 2, NCH], FP8D, name="et", tag="et", bufs=2)
                            for kslot in range(2):
                                kt = 2 * pair + kslot
                                st = at_ps.tile([P, 2, 2, NCH], F32, name="st", tag="st",
                                                bufs=2)
                                score_block(j, 0, kt, st, ktbA, (0, 1))
                                exp_block(j, 0, kt, st, et, kslot, (0, 1))
                            av_block(j, pair, et, vbA, ya, (0, 1),
                                     starts=(pair == 0, False),
                                     stops=(False, False))
                        for h in range(2):
                            div_write(h, j, 0, ya[h][0:D, 0, :], ya[h][D:D + 1, 0, :])
                        # ---- phase B: keys 1024:2048, chunk 1 only ----
                        for pair in range(4):
                            etb = sm.tile([P, 2, 2, 1, NCH], FP8D, name="etb", tag="etb", bufs=2)
                            for kslot in range(2):
                                kt = 2 * pair + kslot
                                stb = at_ps.tile([P, 2, 2, NCH], F32, name="st", tag="st",
                                                 bufs=2)
                                score_block(j, 1, kt, stb, ktbB, (1,))
                                exp_block(j, 1, kt, stb, etb, kslot, (1,))
                            av_block(j, pair, etb, vbB, ya, (1,),
                                     starts=(False,), stops=(pair == 3,))
                        for h in range(2):
                            div_write(h, j, 1, ya[h][0:D, 1, :], ya[h][D:D + 1, 1, :])

                proj([0, 1, 2, 3])

        # ================= LN2 + MLP =================
        with tc.tile_pool(name="mlpp", bufs=1) as mp:
            wmp = wmlp_outer
            ln2 = layer_norm(x2, mp, "ln2", dt=BF16)
            xln2T = transpose_to(ln2, mp, "xln2T", dt=FC1D, idn=ident_bf)
            for t in range(4):
                nc.gpsimd.tensor_tensor(out=x2[:, t, :], in0=x2[:, t, :], in1=b2_bc, op=ALU.add)

            h_sb = mp.tile([P, 32, 512], FC2D)
            for half in range(2):
                with tc.tile_pool(name=f"mlp_ps{half}", bufs=1, space="PSUM") as mlp_ps:
                    ops = [mlp_ps.tile([P, 512], F32, name=f"ops{t}", tag=f"ops{t}", bufs=1)
                           for t in range(4)]
                    for m in range(32):
                        if half == 0:
                            if m % 4 == 0:
                                wfc = wmp.tile([P, 8, 512], FC1D, name="wfc", tag="wfc")
                                nc.sync.dma_start(out=wfc,
                                                    in_=wfc_d[:, 512 * (m // 4):512 * (m // 4 + 1)]
                                                    .rearrange("(kc kp) n -> kp kc n", kp=P))
                            mo = P * (m % 4)
                            fps = mlp_ps.tile([P, 512], F32, name="fps", tag="fps", bufs=4)
                            if FP8_FC1:
                                for k in range(4):
                                    for hh in range(2):
                                        nc.tensor.matmul(fps[:, 256 * hh:256 * (hh + 1)],
                                                         wfc[:, 2 * k:2 * k + 2, mo:mo + P],
                                                         xln2T[:, 2 * k:2 * k + 2, 256 * hh:256 * (hh + 1)],
                                                         start=(k == 0 and hh == 0),
                                                         stop=(k == 3 and hh == 1), perf_mode=DR)
                            else:
                                for k in range(8):
                                    nc.tensor.matmul(fps[:, :], wfc[:, k, mo:mo + P], xln2T[:, k, :],
                                                     start=(k == 0), stop=(k == 7))
                            nc.scalar.activation(out=h_sb[:, m, :], in_=fps[:, :], func=AF.Gelu,
                                                 bias=bfc_sb[:, m:m + 1], scale=1.0)
                        if m % 4 == 0:
                            w2 = wmp.tile([P, 4, 512], FC2D, name="w2", tag="w2", bufs=3)
                            nc.sync.dma_start(out=w2, in_=wfc2_d[P * m:P * (m + 4),
                                                               512 * half:512 * (half + 1)]
                                                .rearrange("(mc mp) n -> mp mc n", mp=P))
                        if FP8_FC2:
                            if m % 2 == 0:
                                for t in range(4):
                                    for n in range(2):
                                        nc.tensor.matmul(
                                            ops[t][:, 256 * n:256 * (n + 1)],
                                            h_sb[:, m:m + 2, P * t:P * (t + 1)],
                                            w2[:, (m % 4):(m % 4) + 2, 256 * n:256 * (n + 1)],
                                            start=(m == 0 and n == 0),
                                            stop=(m == 30 and n == 1), perf_mode=DR)
                        else:
                            for t in range(4):
                                nc.tensor.matmul(ops[t][:, :], h_sb[:, m, P * t:P * (t + 1)],
                                                 w2[:, m % 4, :], start=(m == 0), stop=(m == 31))
                    for t in range(4):
                        nc.vector.tensor_tensor(out=x2[:, t, 512 * half:512 * (half + 1)],
                                                in0=ops[t][:, :],
                                                in1=x2[:, t, 512 * half:512 * (half + 1)], op=ALU.add)
                        if half == 1:
                            (nc.sync if t % 2 == 0 else nc.scalar).dma_start(
                                out=out_ext[P * t:P * (t + 1), :], in_=x2[:, t, :])

    nc.finalize()
    return nc


def _get_nc():
    if "nc" not in _CACHE:
        _CACHE["nc"] = _build()
    return _CACHE["nc"]


def _prep(**inputs):
    f = lambda a: np.asarray(a, dtype=np.float32)
    x = f(inputs["x"])
    ln1_g, ln1_b = f(inputs["ln1_g"]), f(inputs["ln1_b"])
    ln2_g, ln2_b = f(inputs["ln2_g"]), f(inputs["ln2_b"])
    W_attn, b_attn = f(inputs["W_attn"]), f(inputs["b_attn"])
    W_o, b_o = f(inputs["W_o"]), f(inputs["b_o"])
    W_fc, b_fc = f(inputs["W_fc"]), f(inputs["b_fc"])
    W_fc2, b_fc2 = f(inputs["W_fc2"]), f(inputs["b_fc2"])

    # fold LN affine params into the next matmul
    W_attn_e = ln1_g[:, None] * W_attn
    b_attn_e = b_attn + ln1_b @ W_attn
    W_fc_e = ln2_g[:, None] * W_fc
    b_fc_e = b_fc + ln2_b @ W_fc
    # V bias contributes a constant through attention: fold b_v @ W_o into
    # the residual bias (K bias is softmax-invariant and dropped).
    rb = b_o + b_attn_e[2 * C:3 * C] @ W_o

    fc1d = FP8 if FP8_FC1 else ml_dtypes.bfloat16
    fc2d = FP8 if FP8_FC2 else ml_dtypes.bfloat16

    in_maps = []
    for r in range(N_CORES):
        b, p = divmod(r, 4)
        c0, c1 = p, 7 - p
        xs = np.concatenate([x[b, NCH * c0:NCH * (c0 + 1)],
                             x[b, NCH * c1:NCH * (c1 + 1)]], axis=0)
        in_maps.append({
            "x": np.ascontiguousarray(xs),
            "qbase": np.array([[NCH * c0, NCH * c1]], dtype=np.float32),
            "wq": W_attn_e[:, 0:C].astype(FP8),
            "wk": W_attn_e[:, C:2 * C].astype(FP8),
            "wv": W_attn_e[:, 2 * C:3 * C].astype(FP8),
            "bq": b_attn_e[0:C],
            "wo": W_o.astype(FP8), "rb": rb,
            "w_fc": W_fc_e.astype(fc1d), "b_fc": b_fc_e,
            "w_fc2": W_fc2.astype(fc2d), "b_fc2": b_fc2,
        })

    def assemble(results):
        out = np.empty((B, T, C), dtype=np.float32)
        for r in range(N_CORES):
            b, p = divmod(r, 4)
            c0, c1 = p, 7 - p
            o = results[r]["out"]
            out[b, NCH * c0:NCH * (c0 + 1)] = o[0:NCH]
            out[b, NCH * c1:NCH * (c1 + 1)] = o[NCH:TOK]
        return out

    return in_maps, assemble


def kernel(**inputs):
    from concourse.bass_utils import run_bass_kernel_spmd

    in_maps, assemble = _prep(**inputs)
    res = run_bass_kernel_spmd(_get_nc(), in_maps, list(range(N_CORES)))
    return assemble(res.results)
